# revision 12
# baseline (speedup 1.0000x reference)
"""Trainium2 Bass kernel for the 2-layer GRU discriminator
(B=1024, T=63, F=257, H=512  ->  out [1024, 1]).

Strategy (pure data parallelism over batch, 8 cores x 128 batch each):
  - All weights/activations resident in SBUF; x streamed per timestep.
  - State kept as h [b=128 partitions, H free] in bf16; per-step PE
    transposes produce hT (cast to fp8) used as the matmul stationary
    operand, so gate matmuls run with the (static, SBUF-resident) weight
    matrices as the moving operand at N=512 free-dim.
  - Gate matmuls are fp8e4m3 with perf_mode=DoubleRow: K-chunk pairs are
    packed [128, 2, dim] so each matmul contracts 256 rows (2 fp8
    weights/cell), halving PE streaming time. Weights are pre-scaled by
    WSCALE=16 to stay in fp8's normal range; sigmoids/tanh descale via the
    free `scale=` affine of the ACT instruction.
  - L0's x contraction uses its true K: one DoubleRow matmul covers
    features 0..255; the tail (feature 256 + ones row carrying the fused
    input biases) is a K=2 matmul per gate, issued as row-tiled
    (tile_position) matmuls on distinct 32-row groups so the three gate
    tails plus the K=1 bhh0n bias matmul all stream concurrently.
  - L1's four bias rows are likewise a single concurrent row-tiled quad
    of K=1 matmuls instead of four serial N=512 streams.
  - Gate tail uses h' = z*prev - (z-1)*n: one fused scalar_tensor_tensor
    computes (z-1)*n, eliminating the extra sigmoid(-x) ACT op per layer.
  - The x-part matmuls for step t+1 are issued mid-step t (lookahead) so
    the PE stays busy while the serial sigmoid/tanh gate chain runs -- this
    also keeps the PE HAM clock monitor at full rate.
  - The entire MLP head collapses to out[b] = sum_t v[t]*(c_t . dnn_w) + c0
    (v = w3@w2@w1), accumulated across all 63 steps into one PSUM bank by
    M=1 matmuls against the per-step transposed state.
"""
import numpy as np
import ml_dtypes
from contextlib import ExitStack

import concourse.bass as bass
import concourse.tile as tile
from concourse import bacc, mybir
from concourse.bass_utils import run_bass_kernel_spmd

AF = mybir.ActivationFunctionType
OP = mybir.AluOpType
PM = mybir.MatmulPerfMode
F32 = mybir.dt.float32
BF16 = mybir.dt.bfloat16
FP8 = mybir.dt.float8e4
NPBF = ml_dtypes.bfloat16
NPF8 = ml_dtypes.float8_e4m3

B, T, F, H = 1024, 63, 257, 512
NCORES = 8
BC = B // NCORES          # 128 batch per core
G3 = 3 * H                # 1536
NK = H // 128             # 4 hidden chunks
NKQ = NK // 2             # 2 hidden chunk-pairs (DoubleRow)
WSCALE = 16.0             # fp8 weight pre-scale (descaled in sigmoid/tanh)
DSCALE = 4096.0           # head dnn-weight pre-scale (descaled in out copy)


def _dr(ap):
    """[128, 2*X] slice -> [128, 2, X] chunk-pair AP for DoubleRow."""
    return ap.rearrange("p (i b) -> p i b", i=2)


def _build_module():
    nc = bacc.Bacc("TRN2", target_bir_lowering=False, debug=False)

    xT_d = nc.dram_tensor("xT", [T, 128, 2 * BC], FP8, kind="ExternalInput").ap()
    xtl_d = nc.dram_tensor("xtl", [128, T * BC], FP8, kind="ExternalInput").ap()
    wih0_d = nc.dram_tensor("wih0", [128, 2 * G3], FP8, kind="ExternalInput").ap()
    wtl_d = nc.dram_tensor("wtl", [128, 2 * H], FP8, kind="ExternalInput").ap()
    whh0_d = nc.dram_tensor("whh0", [NKQ, 128, 2 * G3], FP8, kind="ExternalInput").ap()
    wih1_d = nc.dram_tensor("wih1", [NKQ, 128, 2 * G3], FP8, kind="ExternalInput").ap()
    whh1_d = nc.dram_tensor("whh1", [NKQ, 128, 2 * G3], FP8, kind="ExternalInput").ap()
    bw_d = nc.dram_tensor("bw", [128, 2 * H], BF16, kind="ExternalInput").ap()
    onesq_d = nc.dram_tensor("onesq", [128, BC], BF16, kind="ExternalInput").ap()
    iden_d = nc.dram_tensor("iden", [128, 128], BF16, kind="ExternalInput").ap()
    dnsc_d = nc.dram_tensor("dnsc", [NKQ, 128, 2 * 64], FP8, kind="ExternalInput").ap()
    out_d = nc.dram_tensor("out", [1, BC], F32, kind="ExternalOutput").ap()

    with tile.TileContext(nc) as tc, ExitStack() as ctx:
        wp = ctx.enter_context(tc.tile_pool(name="wp", bufs=1, space="SBUF"))
        xp = ctx.enter_context(tc.tile_pool(name="xp", bufs=4, space="SBUF"))
        sp = ctx.enter_context(tc.tile_pool(name="sp", bufs=2, space="SBUF"))
        pg = ctx.enter_context(tc.tile_pool(name="pg", bufs=3, space="PSUM"))
        pt = ctx.enter_context(tc.tile_pool(name="pt", bufs=1, space="PSUM"))
        ph = ctx.enter_context(tc.tile_pool(name="ph", bufs=1, space="PSUM"))

        # --- resident weights (chunk-pair packed for DoubleRow) ---
        wih0 = wp.tile_from(wih0_d, name="wih0")
        wtl = wp.tile_from(wtl_d, name="wtl")
        whh0 = [wp.tile_from(whh0_d[q], name=f"whh0_{q}") for q in range(NKQ)]
        wih1 = [wp.tile_from(wih1_d[q], name=f"wih1_{q}") for q in range(NKQ)]
        whh1 = [wp.tile_from(whh1_d[q], name=f"whh1_{q}") for q in range(NKQ)]
        bw = wp.tile_from(bw_d, name="bw")
        onesq = wp.tile_from(onesq_d, name="onesq")
        iden = wp.tile_from(iden_d, name="iden")
        dnsc = [wp.tile_from(dnsc_d[q], name=f"dnsc_{q}") for q in range(NKQ)]
        xtl = wp.tile_from(xtl_d, name="xtl")

        def wslice(wtile, g0, g1):
            """[128, 2*G3] pair tile -> [128, 2, g1-g0] moving operand."""
            return wtile.rearrange("p (i g) -> p i g", i=2)[:, :, g0:g1]

        head = ph.tile([1, BC], F32, name="head", tag="head", bufs=1)

        aT = None
        cT = None
        a_prev = None
        c_prev = None

        def alloc_g(nm, tag):
            pr = pg.tile([BC, H], F32, name=f"pr{nm}", tag=tag)
            pz = pg.tile([BC, H], F32, name=f"pz{nm}", tag=tag)
            phn = pg.tile([BC, H], F32, name=f"phn{nm}", tag=tag)
            pxn = pg.tile([BC, H], F32, name=f"pxn{nm}", tag=tag)
            return pr, pz, phn, pxn

        def dma_x(t):
            xt = xp.tile([128, 2 * BC], FP8, name="xt", tag="xt")
            nc.sync.dma_start(out=xt, in_=xT_d[t])
            return xt

        def l0_xpart(t, g, xt):
            """x-side matmuls for step t into L0 psum gen g=(pr,pz,phn,pxn).
            One DR matmul per gate (features 0..255) + a row-tiled quad:
            K=2 tails (feature 256 + ones->biases) for r/z/n on row groups
            0-2 and the K=1 bhh0n bias reusing group 0."""
            pr, pz, phn, pxn = g
            xq = _dr(xt)
            last = t == 0  # no hidden matmuls at t=0: close groups here
            nc.tensor.matmul(pr, xq, wslice(wih0, 0, H),
                             start=True, stop=False, perf_mode=PM.DoubleRow)
            nc.tensor.matmul(pz, xq, wslice(wih0, H, 2 * H),
                             start=True, stop=False, perf_mode=PM.DoubleRow)
            nc.tensor.matmul(pxn, xq, wslice(wih0, 2 * H, G3),
                             start=True, stop=False, perf_mode=PM.DoubleRow)
            # row groups 0/32/64 only: group 3 (base 96) hits the quadrant-3
            # XBUS HW bug. The K=1 bhh0n bias matmul reuses group 0 (row 0
            # of xtl is ones) and simply serializes after the r tail.
            tb = slice(t * BC, (t + 1) * BC)
            nc.tensor.matmul(pr, xtl[0:2, tb], wtl[0:2, 0:H],
                             start=False, stop=last, tile_position=(0, 0))
            nc.tensor.matmul(pz, xtl[32:34, tb], wtl[32:34, 0:H],
                             start=False, stop=last, tile_position=(32, 0))
            nc.tensor.matmul(pxn, xtl[64:66, tb], wtl[64:66, 0:H],
                             start=False, stop=True, tile_position=(64, 0))
            nc.tensor.matmul(phn, xtl[0:1, tb], wtl[0:1, H:2 * H],
                             start=True, stop=last, tile_position=(0, 0))

        def gru_gates(g, prev, nm):
            """PSUM preacts (x WSCALE) -> new state [BC, H] bf16 in SBUF.
            h' = z*prev - (z-1)*n  (== (1-z)*n + z*prev)."""
            pr, pz, phn, pxn = g
            r = sp.tile([BC, H], BF16, name=f"r_{nm}", tag=f"r_{nm}")
            z = sp.tile([BC, H], BF16, name=f"z_{nm}", tag=f"z_{nm}")
            nn_t = sp.tile([BC, H], BF16, name=f"n_{nm}", tag=f"n_{nm}")
            t3 = sp.tile([BC, H], BF16, name=f"t3_{nm}", tag=f"t3_{nm}")
            t4 = sp.tile([BC, H], BF16, name=f"t4_{nm}", tag=f"t4_{nm}")
            s = sp.tile([BC, H], BF16, name=f"s_{nm}", tag=f"s_{nm}")
            hnew = sp.tile([BC, H], BF16, name=f"h_{nm}", tag=f"h_{nm}")
            nc.scalar.activation(out=r, in_=pr, func=AF.Sigmoid, scale=1.0 / WSCALE)
            nc.scalar.activation(out=z, in_=pz, func=AF.Sigmoid, scale=1.0 / WSCALE)
            nc.vector.tensor_tensor(out=t3, in0=r, in1=phn, op=OP.mult)
            nc.vector.tensor_tensor(out=t4, in0=t3, in1=pxn, op=OP.add)
            if prev is not None:
                u = sp.tile([BC, H], BF16, name=f"u_{nm}", tag=f"u_{nm}")
                nc.vector.tensor_tensor(out=u, in0=z, in1=prev, op=OP.mult)
            nc.scalar.activation(out=nn_t, in_=t4, func=AF.Tanh, scale=1.0 / WSCALE)
            # s = (z-1)*n in one fused DVE op; h' = u - s
            nc.vector.scalar_tensor_tensor(out=s, in0=z, scalar=1.0, in1=nn_t,
                                           op0=OP.subtract, op1=OP.mult)
            if prev is None:
                nc.vector.tensor_scalar(out=hnew, in0=s, scalar1=-1.0,
                                        scalar2=None, op0=OP.mult)
            else:
                nc.vector.tensor_tensor(out=hnew, in0=u, in1=s, op=OP.subtract)
            return hnew

        def transpose_state(h, nm):
            """[BC, H] SBUF bf16 -> [128, H] SBUF fp8 holding hT chunks.
            Copy per chunk-pair so the first DoubleRow stationary is ready
            before the whole transpose finishes; split engines by state."""
            ptr = pt.tile([128, H], F32, name=f"ptr_{nm}", tag="tr")
            for k in range(NK):
                # regular matmul h_chunk^T @ I == transpose, but unlike
                # transpose-mode it counts as PE-busy for the HAM clock
                # monitor, keeping the array at 2.4 GHz through the gate
                # chains (transpose-mode time reads as idle and cools it)
                nc.tensor.matmul(
                    ptr[:, k * 128:(k + 1) * 128],
                    h[:, k * 128:(k + 1) * 128],
                    iden,
                    start=True, stop=True,
                )
            hT = sp.tile([128, H], FP8, name=f"hT_{nm}", tag=f"hT_{nm}")
            for q in range(2):
                sl = slice(q * 256, (q + 1) * 256)
                if nm == "a":
                    nc.scalar.activation(out=hT[:, sl], in_=ptr[:, sl], func=AF.Copy)
                else:
                    nc.vector.tensor_copy(out=hT[:, sl], in_=ptr[:, sl])
            return hT

        # ---- prologue: x-side for t=0, DMA lookahead for t=1 ----
        g0 = alloc_g("0", "g0")
        xt_cur = dma_x(0)
        xt_next = dma_x(1)
        l0_xpart(0, g0, xt_cur)

        for t in range(T):
            # ---- A: chain-critical L0 hidden matmuls (need aT(t-1)) ----
            # gate-major, r first then n (t3 needs phn early) then z
            if t > 0:
                for gs, pdst in ((0, g0[0]), (2 * H, g0[2]), (H, g0[1])):
                    for q in range(NKQ):
                        aq = _dr(aT[:, 2 * q * 128:(2 * q + 2) * 128])
                        nc.tensor.matmul(pdst, aq, wslice(whh0[q], gs, gs + H),
                                         start=False, stop=(q == NKQ - 1),
                                         perf_mode=PM.DoubleRow)

            # ---- B: filler while the a-chain runs ----
            g1 = alloc_g("1", "g1")
            nc.tensor.matmul(g1[0], onesq[0:1, :], bw[0:1, 0:H],
                             start=True, stop=False, tile_position=(0, 0))
            nc.tensor.matmul(g1[1], onesq[32:33, :], bw[32:33, 0:H],
                             start=True, stop=False, tile_position=(32, 0))
            nc.tensor.matmul(g1[2], onesq[64:65, :], bw[64:65, 0:H],
                             start=True, stop=(t == 0), tile_position=(64, 0))
            nc.tensor.matmul(g1[3], onesq[0:1, :], bw[0:1, H:2 * H],
                             start=True, stop=False, tile_position=(0, 0))
            if t > 0:
                # L1 hidden (cT-dependent): r, n, z; phn1 closes here
                for gs, pdst, st in ((0, g1[0], False), (2 * H, g1[2], True),
                                     (H, g1[1], False)):
                    for q in range(NKQ):
                        cq = _dr(cT[:, 2 * q * 128:(2 * q + 2) * 128])
                        nc.tensor.matmul(pdst, cq, wslice(whh1[q], gs, gs + H),
                                         start=False, stop=(st and q == NKQ - 1),
                                         perf_mode=PM.DoubleRow)
                # head accumulation for step t-1 (cT long ready)
                for q in range(NKQ):
                    hl = dnsc[q].rearrange("p (i t) -> p i t", i=2)[:, :, t - 1:t]
                    nc.tensor.matmul(head, hl,
                                     _dr(cT[:, 2 * q * 128:(2 * q + 2) * 128]),
                                     start=(t == 1 and q == 0), stop=False,
                                     perf_mode=PM.DoubleRow)

            # ---- C: layer-0 gate chain (ACT/DVE) ----
            a_new = gru_gates(g0, a_prev, "a")
            a_prev = a_new

            # ---- D: lookahead — L0 x-side for t+1 fills the PE while the
            # a-chain finishes; DMA for t+2 stays a full step ahead ----
            if t + 1 < T:
                if t + 2 < T:
                    xt_cur, xt_next = xt_next, dma_x(t + 2)
                else:
                    xt_cur = xt_next
                g0 = alloc_g("0", "g0")
                l0_xpart(t + 1, g0, xt_cur)

            # ---- E: transpose a ----
            aT = transpose_state(a_new, "a")

            # ---- F: L1 a-side matmuls; pxn1 closes last ----
            for gs, pdst in ((0, g1[0]), (H, g1[1]), (2 * H, g1[3])):
                for q in range(NKQ):
                    aq = _dr(aT[:, 2 * q * 128:(2 * q + 2) * 128])
                    nc.tensor.matmul(pdst, aq, wslice(wih1[q], gs, gs + H),
                                     start=False, stop=(q == NKQ - 1),
                                     perf_mode=PM.DoubleRow)

            # ---- G: layer-1 gate chain ----
            c_new = gru_gates(g1, c_prev, "c")
            c_prev = c_new

            # ---- H: transpose c ----
            cT = transpose_state(c_new, "c")

        for q in range(NKQ):
            hl = dnsc[q].rearrange("p (i t) -> p i t", i=2)[:, :, T - 1:T]
            nc.tensor.matmul(head, hl,
                             _dr(cT[:, 2 * q * 128:(2 * q + 2) * 128]),
                             start=False, stop=(q == NKQ - 1),
                             perf_mode=PM.DoubleRow)
        out_sb = sp.tile([1, BC], F32, name="out_sb", tag="out_sb")
        nc.scalar.activation(out=out_sb, in_=head, func=AF.Copy, scale=1.0 / DSCALE)
        nc.sync.dma_start(out=out_d, in_=out_sb)

    # legalize sem waits (>=2 waits per matmul is a codegen error) etc.
    nc.compile()
    return nc


def _pack_pairs(wt):
    """[512, G3] (contraction-major) -> [NQ, 128, 2*G3] chunk-pair tiles:
    out[q][p, i*G3+g] = wt[(2q+i)*128 + p, g]"""
    nq = wt.shape[0] // 256
    return np.ascontiguousarray(
        wt.reshape(nq, 2, 128, -1).transpose(0, 2, 1, 3).reshape(nq, 128, -1))


def host_prep(inputs):
    f32 = np.float32
    x = np.asarray(inputs["x"], f32)
    w_ih0, w_hh0 = np.asarray(inputs["w_ih0"], f32), np.asarray(inputs["w_hh0"], f32)
    b_ih0, b_hh0 = np.asarray(inputs["b_ih0"], f32), np.asarray(inputs["b_hh0"], f32)
    w_ih1, w_hh1 = np.asarray(inputs["w_ih1"], f32), np.asarray(inputs["w_hh1"], f32)
    b_ih1, b_hh1 = np.asarray(inputs["b_ih1"], f32), np.asarray(inputs["b_hh1"], f32)
    dnn_w, dnn_b = np.asarray(inputs["dnn_w"], f32), np.asarray(inputs["dnn_b"], f32)
    w1, b1 = np.asarray(inputs["w1"], f32), np.asarray(inputs["b1"], f32)
    w2, b2 = np.asarray(inputs["w2"], f32), np.asarray(inputs["b2"], f32)
    w3, b3 = np.asarray(inputs["w3"], f32), np.asarray(inputs["b3"], f32)

    # L0 input weights: features 0..255 as one DoubleRow pair chunk; the
    # tail tile wtl carries feature 256 (row 0 of each pair) and the fused
    # biases (row 1): b_ih0+b_hh0 for r/z, b_ih0 for n; plus bhh0n at row 96.
    wihT = w_ih0.T * WSCALE                      # [F=257, G3]
    biasrow = np.concatenate([(b_ih0 + b_hh0)[:2 * H], b_ih0[2 * H:]]) * WSCALE
    wih0 = _pack_pairs(wihT[:256])[0].astype(NPF8)   # [128, 2*G3]
    # tail moving tile: row base+0 pairs with the ones row of xtl (biases),
    # row base+1 with the x256 row; cols H:2H row 0 carries bhh0n (K=1)
    wtl = np.zeros((128, 2 * H), f32)
    for gi, base in enumerate((0, 32, 64)):
        wtl[base, :H] = biasrow[gi * H:(gi + 1) * H]
        wtl[base + 1, :H] = wihT[256, gi * H:(gi + 1) * H]
    wtl[0, H:] = b_hh0[2 * H:] * WSCALE
    wtl = wtl.astype(NPF8)

    whh0 = _pack_pairs(w_hh0.T * WSCALE).astype(NPF8)
    wih1 = _pack_pairs(w_ih1.T * WSCALE).astype(NPF8)
    whh1 = _pack_pairs(w_hh1.T * WSCALE).astype(NPF8)

    # L1 bias rows for the row-tiled quad: b1r/b1z (=b_ih1+b_hh1), bhh1n, bih1n
    b1g = b_ih1 + b_hh1
    bw = np.zeros((128, 2 * H), f32)
    bw[0, :H] = b1g[:H]
    bw[32, :H] = b1g[H:2 * H]
    bw[64, :H] = b_hh1[2 * H:]
    bw[0, H:] = b_ih1[2 * H:]
    bw = (bw * WSCALE).astype(NPBF)

    v = (w3 @ w2 @ w1)[0]
    # chunk-pair packed for DoubleRow, inner dim padded 63->64 so the
    # pair-dim byte step (64) satisfies the fp8-DR step%16==0 ISA rule
    dfull = np.zeros((H, 64), f32)
    dfull[:, :T] = dnn_w[0][:, None] * v[None, :] * DSCALE
    dnsc = np.ascontiguousarray(
        dfull.reshape(NKQ, 2, 128, 64).transpose(0, 2, 1, 3)
        .reshape(NKQ, 128, 2 * 64)).astype(NPF8)
    c_all = float(v.sum() * dnn_b[0] + (w3 @ w2 @ b1)[0] + (w3 @ b2)[0] + b3[0])

    shared = dict(
        wih0=wih0, wtl=wtl, whh0=whh0, wih1=wih1, whh1=whh1, bw=bw,
        onesq=np.ones((128, BC), NPBF), iden=np.eye(128, dtype=NPBF), dnsc=dnsc)

    percore = []
    for c in range(NCORES):
        xc = x[c * BC:(c + 1) * BC]              # [BC, T, F]
        xmain = xc[:, :, :256]                   # [BC, T, 256]
        xT = (xmain.reshape(BC, T, 2, 128).transpose(1, 3, 2, 0)
              .reshape(T, 128, 2 * BC))
        xtl = np.zeros((128, T * BC), f32)
        x256 = xc[:, :, 256].T.reshape(T * BC)   # [T*BC] time-major
        for base in (0, 32, 64):
            xtl[base] = 1.0
            xtl[base + 1] = x256
        percore.append({"xT": np.ascontiguousarray(xT).astype(NPF8),
                        "xtl": xtl.astype(NPF8)})
    return shared, percore, c_all


_CACHED = {}


def _get_module():
    if "nc" not in _CACHED:
        _CACHED["nc"] = _build_module()
    return _CACHED["nc"]


def kernel(**inputs) -> np.ndarray:
    shared, percore, c_all = host_prep(inputs)
    nc = _get_module()
    in_maps = [{**shared, **percore[c]} for c in range(NCORES)]
    res = run_bass_kernel_spmd(nc, in_maps, core_ids=list(range(NCORES)))
    outs = [res.results[c]["out"].reshape(BC) for c in range(NCORES)]
    out = np.concatenate(outs).astype(np.float32) + np.float32(c_all)
    return out.reshape(B, 1)


# revision 14
# speedup vs baseline: 1.1369x; 1.1369x over previous
"""Trainium2 Bass kernel for the 2-layer GRU discriminator
(B=1024, T=63, F=257, H=512  ->  out [1024, 1]).

Strategy (pure data parallelism over batch, 8 cores x 128 batch each):
  - All weights/activations resident in SBUF; x streamed per timestep.
  - State kept as h [b=128 partitions, H free] in bf16; per-step PE
    transposes produce hT (cast to fp8) used as the matmul stationary
    operand, so gate matmuls run with the (static, SBUF-resident) weight
    matrices as the moving operand at N=512 free-dim.
  - Gate matmuls are fp8e4m3 with perf_mode=DoubleRow: K-chunk pairs are
    packed [128, 2, dim] so each matmul contracts 256 rows (2 fp8
    weights/cell), halving PE streaming time. Weights are pre-scaled by
    WSCALE=16 to stay in fp8's normal range; sigmoids/tanh descale via the
    free `scale=` affine of the ACT instruction.
  - L0's x contraction uses its true K: one DoubleRow matmul covers
    features 0..255; the tail (feature 256 + ones row carrying the fused
    input biases) is a K=2 matmul per gate, issued as row-tiled
    (tile_position) matmuls on distinct 32-row groups so the three gate
    tails plus the K=1 bhh0n bias matmul all stream concurrently.
  - L1's four bias rows are likewise a single concurrent row-tiled quad
    of K=1 matmuls instead of four serial N=512 streams.
  - Gate tail uses h' = z*prev - (z-1)*n: one fused scalar_tensor_tensor
    computes (z-1)*n, eliminating the extra sigmoid(-x) ACT op per layer.
  - The x-part matmuls for step t+1 are issued mid-step t (lookahead) so
    the PE stays busy while the serial sigmoid/tanh gate chain runs -- this
    also keeps the PE HAM clock monitor at full rate.
  - The entire MLP head collapses to out[b] = sum_t v[t]*(c_t . dnn_w) + c0
    (v = w3@w2@w1), accumulated across all 63 steps into one PSUM bank by
    M=1 matmuls against the per-step transposed state.
"""
import numpy as np
import ml_dtypes
from contextlib import ExitStack

import concourse.bass as bass
import concourse.tile as tile
from concourse import bacc, mybir
from concourse.bass_utils import run_bass_kernel_spmd

AF = mybir.ActivationFunctionType
OP = mybir.AluOpType
PM = mybir.MatmulPerfMode
F32 = mybir.dt.float32
BF16 = mybir.dt.bfloat16
FP8 = mybir.dt.float8e4
NPBF = ml_dtypes.bfloat16
NPF8 = ml_dtypes.float8_e4m3

B, T, F, H = 1024, 63, 257, 512
NCORES = 8
BC = B // NCORES          # 128 batch per core
G3 = 3 * H                # 1536
NK = H // 128             # 4 hidden chunks
NKQ = NK // 2             # 2 hidden chunk-pairs (DoubleRow)
WSCALE = 16.0             # fp8 weight pre-scale (descaled in sigmoid/tanh)
DSCALE = 4096.0           # head dnn-weight pre-scale (descaled in out copy)


def _dr(ap):
    """[128, 2*X] slice -> [128, 2, X] chunk-pair AP for DoubleRow."""
    return ap.rearrange("p (i b) -> p i b", i=2)


def _build_module():
    nc = bacc.Bacc("TRN2", target_bir_lowering=False, debug=False)

    xT_d = nc.dram_tensor("xT", [T, 128, 2 * BC], FP8, kind="ExternalInput").ap()
    xtl_d = nc.dram_tensor("xtl", [128, T * BC], FP8, kind="ExternalInput").ap()
    wih0_d = nc.dram_tensor("wih0", [128, 2 * G3], FP8, kind="ExternalInput").ap()
    wtl_d = nc.dram_tensor("wtl", [128, 2 * H], FP8, kind="ExternalInput").ap()
    whh0_d = nc.dram_tensor("whh0", [NKQ, 128, 2 * G3], FP8, kind="ExternalInput").ap()
    wih1_d = nc.dram_tensor("wih1", [NKQ, 128, 2 * G3], FP8, kind="ExternalInput").ap()
    whh1_d = nc.dram_tensor("whh1", [NKQ, 128, 2 * G3], FP8, kind="ExternalInput").ap()
    bw_d = nc.dram_tensor("bw", [128, 2 * H], BF16, kind="ExternalInput").ap()
    onesq_d = nc.dram_tensor("onesq", [128, BC], BF16, kind="ExternalInput").ap()
    iden_d = nc.dram_tensor("iden", [128, 128], BF16, kind="ExternalInput").ap()
    dnsc_d = nc.dram_tensor("dnsc", [NKQ, 128, 2 * 64], FP8, kind="ExternalInput").ap()
    out_d = nc.dram_tensor("out", [1, BC], F32, kind="ExternalOutput").ap()

    with tile.TileContext(nc) as tc, ExitStack() as ctx:
        wp = ctx.enter_context(tc.tile_pool(name="wp", bufs=1, space="SBUF"))
        xp = ctx.enter_context(tc.tile_pool(name="xp", bufs=4, space="SBUF"))
        sp = ctx.enter_context(tc.tile_pool(name="sp", bufs=2, space="SBUF"))
        pg = ctx.enter_context(tc.tile_pool(name="pg", bufs=3, space="PSUM"))
        pt = ctx.enter_context(tc.tile_pool(name="pt", bufs=1, space="PSUM"))
        ph = ctx.enter_context(tc.tile_pool(name="ph", bufs=1, space="PSUM"))

        # --- resident weights (chunk-pair packed for DoubleRow) ---
        wih0 = wp.tile_from(wih0_d, name="wih0")
        wtl = wp.tile_from(wtl_d, name="wtl")
        whh0 = [wp.tile_from(whh0_d[q], name=f"whh0_{q}") for q in range(NKQ)]
        wih1 = [wp.tile_from(wih1_d[q], name=f"wih1_{q}") for q in range(NKQ)]
        whh1 = [wp.tile_from(whh1_d[q], name=f"whh1_{q}") for q in range(NKQ)]
        bw = wp.tile_from(bw_d, name="bw")
        onesq = wp.tile_from(onesq_d, name="onesq")
        iden = wp.tile_from(iden_d, name="iden")
        dnsc = [wp.tile_from(dnsc_d[q], name=f"dnsc_{q}") for q in range(NKQ)]
        xtl = wp.tile_from(xtl_d, name="xtl")

        def wslice(wtile, g0, g1):
            """[128, 2*G3] pair tile -> [128, 2, g1-g0] moving operand."""
            return wtile.rearrange("p (i g) -> p i g", i=2)[:, :, g0:g1]

        head = ph.tile([1, BC], F32, name="head", tag="head", bufs=1)

        aT = None
        cT = None
        a_prev = None
        c_prev = None

        def alloc_g(nm, tag):
            pr = pg.tile([BC, H], F32, name=f"pr{nm}", tag=tag)
            pz = pg.tile([BC, H], F32, name=f"pz{nm}", tag=tag)
            phn = pg.tile([BC, H], F32, name=f"phn{nm}", tag=tag)
            pxn = pg.tile([BC, H], F32, name=f"pxn{nm}", tag=tag)
            return pr, pz, phn, pxn

        def dma_x(t):
            xt = xp.tile([128, 2 * BC], FP8, name="xt", tag="xt")
            nc.sync.dma_start(out=xt, in_=xT_d[t])
            return xt

        def l0_xpart(t, g, xt):
            """x-side matmuls for step t into L0 psum gen g=(pr,pz,phn,pxn).
            One DR matmul per gate (features 0..255) + a row-tiled quad:
            K=2 tails (feature 256 + ones->biases) for r/z/n on row groups
            0-2 and the K=1 bhh0n bias reusing group 0."""
            pr, pz, phn, pxn = g
            xq = _dr(xt)
            last = t == 0  # no hidden matmuls at t=0: close groups here
            nc.tensor.matmul(pr, xq, wslice(wih0, 0, H),
                             start=True, stop=False, perf_mode=PM.DoubleRow)
            nc.tensor.matmul(pz, xq, wslice(wih0, H, 2 * H),
                             start=True, stop=False, perf_mode=PM.DoubleRow)
            nc.tensor.matmul(pxn, xq, wslice(wih0, 2 * H, G3),
                             start=True, stop=False, perf_mode=PM.DoubleRow)
            # row groups 0/32/64 only: group 3 (base 96) hits the quadrant-3
            # XBUS HW bug. The K=1 bhh0n bias matmul reuses group 0 (row 0
            # of xtl is ones) and simply serializes after the r tail.
            tb = slice(t * BC, (t + 1) * BC)
            nc.tensor.matmul(pr, xtl[0:2, tb], wtl[0:2, 0:H],
                             start=False, stop=last, tile_position=(0, 0))
            nc.tensor.matmul(pz, xtl[32:34, tb], wtl[32:34, 0:H],
                             start=False, stop=last, tile_position=(32, 0))
            nc.tensor.matmul(pxn, xtl[64:66, tb], wtl[64:66, 0:H],
                             start=False, stop=True, tile_position=(64, 0))
            nc.tensor.matmul(phn, xtl[0:1, tb], wtl[0:1, H:2 * H],
                             start=True, stop=last, tile_position=(0, 0))

        def gru_gates(g, prev, nm):
            """PSUM preacts (x WSCALE) -> new state [BC, H] bf16 in SBUF.
            h' = z*prev - (z-1)*n  (== (1-z)*n + z*prev)."""
            pr, pz, phn, pxn = g
            r = sp.tile([BC, H], BF16, name=f"r_{nm}", tag=f"r_{nm}")
            z = sp.tile([BC, H], BF16, name=f"z_{nm}", tag=f"z_{nm}")
            nn_t = sp.tile([BC, H], BF16, name=f"n_{nm}", tag=f"n_{nm}")
            t3 = sp.tile([BC, H], BF16, name=f"t3_{nm}", tag=f"t3_{nm}")
            t4 = sp.tile([BC, H], BF16, name=f"t4_{nm}", tag=f"t4_{nm}")
            s = sp.tile([BC, H], BF16, name=f"s_{nm}", tag=f"s_{nm}")
            hnew = sp.tile([BC, H], BF16, name=f"h_{nm}", tag=f"h_{nm}")
            nc.scalar.activation(out=r, in_=pr, func=AF.Sigmoid, scale=1.0 / WSCALE)
            nc.scalar.activation(out=z, in_=pz, func=AF.Sigmoid, scale=1.0 / WSCALE)
            nc.vector.tensor_tensor(out=t3, in0=r, in1=phn, op=OP.mult)
            nc.vector.tensor_tensor(out=t4, in0=t3, in1=pxn, op=OP.add)
            if prev is not None:
                u = sp.tile([BC, H], BF16, name=f"u_{nm}", tag=f"u_{nm}")
                nc.vector.tensor_tensor(out=u, in0=z, in1=prev, op=OP.mult)
            nc.scalar.activation(out=nn_t, in_=t4, func=AF.Tanh, scale=1.0 / WSCALE)
            # s = (z-1)*n in one fused DVE op; h' = u - s
            nc.vector.scalar_tensor_tensor(out=s, in0=z, scalar=1.0, in1=nn_t,
                                           op0=OP.subtract, op1=OP.mult)
            if prev is None:
                nc.vector.tensor_scalar(out=hnew, in0=s, scalar1=-1.0,
                                        scalar2=None, op0=OP.mult)
            else:
                nc.vector.tensor_tensor(out=hnew, in0=u, in1=s, op=OP.subtract)
            return hnew

        def transpose_state(h, nm):
            """[BC, H] SBUF bf16 -> [128, H] SBUF fp8 holding hT chunks.
            Copy per chunk-pair so the first DoubleRow stationary is ready
            before the whole transpose finishes; split engines by state."""
            ptr = pt.tile([128, H], F32, name=f"ptr_{nm}", tag="tr")
            for k in range(NK):
                # regular matmul h_chunk^T @ I == transpose, but unlike
                # transpose-mode it counts as PE-busy for the HAM clock
                # monitor, keeping the array at 2.4 GHz through the gate
                # chains (transpose-mode time reads as idle and cools it)
                nc.tensor.matmul(
                    ptr[:, k * 128:(k + 1) * 128],
                    h[:, k * 128:(k + 1) * 128],
                    iden,
                    start=True, stop=True,
                )
            hT = sp.tile([128, H], FP8, name=f"hT_{nm}", tag=f"hT_{nm}")
            for q in range(2):
                sl = slice(q * 256, (q + 1) * 256)
                if nm == "a":
                    nc.scalar.activation(out=hT[:, sl], in_=ptr[:, sl], func=AF.Copy)
                else:
                    nc.vector.tensor_copy(out=hT[:, sl], in_=ptr[:, sl])
            return hT

        def l1_aside(g1, aT):
            """L1 x-side (a-state) matmuls; closes pr1/pz1/pxn1."""
            for gs, pdst in ((0, g1[0]), (H, g1[1]), (2 * H, g1[3])):
                for q in range(NKQ):
                    aq = _dr(aT[:, 2 * q * 128:(2 * q + 2) * 128])
                    nc.tensor.matmul(pdst, aq, wslice(wih1[q], gs, gs + H),
                                     start=False, stop=(q == NKQ - 1),
                                     perf_mode=PM.DoubleRow)

        def l1_fill(t, cT):
            """g1(t) alloc + bias quad + L1 hidden (cT(t-1)); phn1 closes."""
            g1 = alloc_g("1", "g1")
            nc.tensor.matmul(g1[0], onesq[0:1, :], bw[0:1, 0:H],
                             start=True, stop=False, tile_position=(0, 0))
            nc.tensor.matmul(g1[1], onesq[32:33, :], bw[32:33, 0:H],
                             start=True, stop=False, tile_position=(32, 0))
            nc.tensor.matmul(g1[2], onesq[64:65, :], bw[64:65, 0:H],
                             start=True, stop=(t == 0), tile_position=(64, 0))
            nc.tensor.matmul(g1[3], onesq[0:1, :], bw[0:1, H:2 * H],
                             start=True, stop=False, tile_position=(0, 0))
            if t > 0:
                for gs, pdst, st in ((0, g1[0], False), (2 * H, g1[2], True),
                                     (H, g1[1], False)):
                    for q in range(NKQ):
                        cq = _dr(cT[:, 2 * q * 128:(2 * q + 2) * 128])
                        nc.tensor.matmul(pdst, cq, wslice(whh1[q], gs, gs + H),
                                         start=False, stop=(st and q == NKQ - 1),
                                         perf_mode=PM.DoubleRow)
            return g1

        def head_mms(t, cT, stop=False):
            for q in range(NKQ):
                hl = dnsc[q].rearrange("p (i t) -> p i t", i=2)[:, :, t:t + 1]
                nc.tensor.matmul(head, hl,
                                 _dr(cT[:, 2 * q * 128:(2 * q + 2) * 128]),
                                 start=(t == 0 and q == 0),
                                 stop=(stop and q == NKQ - 1),
                                 perf_mode=PM.DoubleRow)

        # ---- prologue: x-side for t=0, DMA lookahead for t=1; L1 gen-0
        # banks + biases ----
        g0 = alloc_g("0", "g0")
        xt_cur = dma_x(0)
        xt_next = dma_x(1)
        l0_xpart(0, g0, xt_cur)
        g1 = l1_fill(0, None)   # banks for the pending layer-1 step (t-1)

        # Software-pipelined steady state: iteration t handles the layer-0
        # chain for step t and the layer-1 chain for step t-1 — their ops
        # interleave in true ready-order on every engine, so neither chain
        # ever waits behind the other's tail in an engine queue.
        for t in range(T):
            # ---- A: L0 hidden for t (needs aT(t-1)) — r, n, z order ----
            if t > 0:
                for gs, pdst in ((0, g0[0]), (2 * H, g0[2]), (H, g0[1])):
                    for q in range(NKQ):
                        aq = _dr(aT[:, 2 * q * 128:(2 * q + 2) * 128])
                        nc.tensor.matmul(pdst, aq, wslice(whh0[q], gs, gs + H),
                                         start=False, stop=(q == NKQ - 1),
                                         perf_mode=PM.DoubleRow)
                # ---- F: L1 a-side for t-1 (same aT(t-1) dependency) ----
                l1_aside(g1, aT)

            # ---- C: both gate chains, interleaved by issue order ----
            a_new = gru_gates(g0, a_prev, "a")
            a_prev = a_new
            if t > 0:
                c_new = gru_gates(g1, c_prev, "c")
                c_prev = c_new

            # ---- D: lookahead — L0 x-side for t+1 fills the PE while the
            # chains run; DMA for t+2 stays a full step ahead ----
            if t + 1 < T:
                if t + 2 < T:
                    xt_cur, xt_next = xt_next, dma_x(t + 2)
                else:
                    xt_cur = xt_next
                g0 = alloc_g("0", "g0")
                l0_xpart(t + 1, g0, xt_cur)

            # ---- E: transpose a(t) ----
            aT = transpose_state(a_new, "a")

            if t > 0:
                # ---- H: transpose c(t-1) ----
                cT = transpose_state(c_new, "c")
                # ---- B: L1 gen-t banks: bias quad + cT(t-1) hidden ----
                g1 = l1_fill(t, cT)
                # ---- head for step t-1 ----
                head_mms(t - 1, cT)

        # ---- epilogue: finish layer 1 for step T-1 ----
        l1_aside(g1, aT)
        c_new = gru_gates(g1, c_prev, "c")
        cT = transpose_state(c_new, "c")
        head_mms(T - 1, cT, stop=True)
        out_sb = sp.tile([1, BC], F32, name="out_sb", tag="out_sb")
        nc.scalar.activation(out=out_sb, in_=head, func=AF.Copy, scale=1.0 / DSCALE)
        nc.sync.dma_start(out=out_d, in_=out_sb)

    # legalize sem waits (>=2 waits per matmul is a codegen error) etc.
    nc.compile()
    return nc


def _pack_pairs(wt):
    """[512, G3] (contraction-major) -> [NQ, 128, 2*G3] chunk-pair tiles:
    out[q][p, i*G3+g] = wt[(2q+i)*128 + p, g]"""
    nq = wt.shape[0] // 256
    return np.ascontiguousarray(
        wt.reshape(nq, 2, 128, -1).transpose(0, 2, 1, 3).reshape(nq, 128, -1))


def host_prep(inputs):
    f32 = np.float32
    x = np.asarray(inputs["x"], f32)
    w_ih0, w_hh0 = np.asarray(inputs["w_ih0"], f32), np.asarray(inputs["w_hh0"], f32)
    b_ih0, b_hh0 = np.asarray(inputs["b_ih0"], f32), np.asarray(inputs["b_hh0"], f32)
    w_ih1, w_hh1 = np.asarray(inputs["w_ih1"], f32), np.asarray(inputs["w_hh1"], f32)
    b_ih1, b_hh1 = np.asarray(inputs["b_ih1"], f32), np.asarray(inputs["b_hh1"], f32)
    dnn_w, dnn_b = np.asarray(inputs["dnn_w"], f32), np.asarray(inputs["dnn_b"], f32)
    w1, b1 = np.asarray(inputs["w1"], f32), np.asarray(inputs["b1"], f32)
    w2, b2 = np.asarray(inputs["w2"], f32), np.asarray(inputs["b2"], f32)
    w3, b3 = np.asarray(inputs["w3"], f32), np.asarray(inputs["b3"], f32)

    # L0 input weights: features 0..255 as one DoubleRow pair chunk; the
    # tail tile wtl carries feature 256 (row 0 of each pair) and the fused
    # biases (row 1): b_ih0+b_hh0 for r/z, b_ih0 for n; plus bhh0n at row 96.
    wihT = w_ih0.T * WSCALE                      # [F=257, G3]
    biasrow = np.concatenate([(b_ih0 + b_hh0)[:2 * H], b_ih0[2 * H:]]) * WSCALE
    wih0 = _pack_pairs(wihT[:256])[0].astype(NPF8)   # [128, 2*G3]
    # tail moving tile: row base+0 pairs with the ones row of xtl (biases),
    # row base+1 with the x256 row; cols H:2H row 0 carries bhh0n (K=1)
    wtl = np.zeros((128, 2 * H), f32)
    for gi, base in enumerate((0, 32, 64)):
        wtl[base, :H] = biasrow[gi * H:(gi + 1) * H]
        wtl[base + 1, :H] = wihT[256, gi * H:(gi + 1) * H]
    wtl[0, H:] = b_hh0[2 * H:] * WSCALE
    wtl = wtl.astype(NPF8)

    whh0 = _pack_pairs(w_hh0.T * WSCALE).astype(NPF8)
    wih1 = _pack_pairs(w_ih1.T * WSCALE).astype(NPF8)
    whh1 = _pack_pairs(w_hh1.T * WSCALE).astype(NPF8)

    # L1 bias rows for the row-tiled quad: b1r/b1z (=b_ih1+b_hh1), bhh1n, bih1n
    b1g = b_ih1 + b_hh1
    bw = np.zeros((128, 2 * H), f32)
    bw[0, :H] = b1g[:H]
    bw[32, :H] = b1g[H:2 * H]
    bw[64, :H] = b_hh1[2 * H:]
    bw[0, H:] = b_ih1[2 * H:]
    bw = (bw * WSCALE).astype(NPBF)

    v = (w3 @ w2 @ w1)[0]
    # chunk-pair packed for DoubleRow, inner dim padded 63->64 so the
    # pair-dim byte step (64) satisfies the fp8-DR step%16==0 ISA rule
    dfull = np.zeros((H, 64), f32)
    dfull[:, :T] = dnn_w[0][:, None] * v[None, :] * DSCALE
    dnsc = np.ascontiguousarray(
        dfull.reshape(NKQ, 2, 128, 64).transpose(0, 2, 1, 3)
        .reshape(NKQ, 128, 2 * 64)).astype(NPF8)
    c_all = float(v.sum() * dnn_b[0] + (w3 @ w2 @ b1)[0] + (w3 @ b2)[0] + b3[0])

    shared = dict(
        wih0=wih0, wtl=wtl, whh0=whh0, wih1=wih1, whh1=whh1, bw=bw,
        onesq=np.ones((128, BC), NPBF), iden=np.eye(128, dtype=NPBF), dnsc=dnsc)

    percore = []
    for c in range(NCORES):
        xc = x[c * BC:(c + 1) * BC]              # [BC, T, F]
        xmain = xc[:, :, :256]                   # [BC, T, 256]
        xT = (xmain.reshape(BC, T, 2, 128).transpose(1, 3, 2, 0)
              .reshape(T, 128, 2 * BC))
        xtl = np.zeros((128, T * BC), f32)
        x256 = xc[:, :, 256].T.reshape(T * BC)   # [T*BC] time-major
        for base in (0, 32, 64):
            xtl[base] = 1.0
            xtl[base + 1] = x256
        percore.append({"xT": np.ascontiguousarray(xT).astype(NPF8),
                        "xtl": xtl.astype(NPF8)})
    return shared, percore, c_all


_CACHED = {}


def _get_module():
    if "nc" not in _CACHED:
        _CACHED["nc"] = _build_module()
    return _CACHED["nc"]


def kernel(**inputs) -> np.ndarray:
    shared, percore, c_all = host_prep(inputs)
    nc = _get_module()
    in_maps = [{**shared, **percore[c]} for c in range(NCORES)]
    res = run_bass_kernel_spmd(nc, in_maps, core_ids=list(range(NCORES)))
    outs = [res.results[c]["out"].reshape(BC) for c in range(NCORES)]
    out = np.concatenate(outs).astype(np.float32) + np.float32(c_all)
    return out.reshape(B, 1)


# revision 16
# speedup vs baseline: 1.1464x; 1.0084x over previous
"""Trainium2 Bass kernel for the 2-layer GRU discriminator
(B=1024, T=63, F=257, H=512  ->  out [1024, 1]).

Strategy (pure data parallelism over batch, 8 cores x 128 batch each):
  - All weights/activations resident in SBUF; x streamed per timestep.
  - State kept as h [b=128 partitions, H free] in bf16; per-step PE
    transposes produce hT (cast to fp8) used as the matmul stationary
    operand, so gate matmuls run with the (static, SBUF-resident) weight
    matrices as the moving operand at N=512 free-dim.
  - Gate matmuls are fp8e4m3 with perf_mode=DoubleRow: K-chunk pairs are
    packed [128, 2, dim] so each matmul contracts 256 rows (2 fp8
    weights/cell), halving PE streaming time. Weights are pre-scaled by
    WSCALE=16 to stay in fp8's normal range; sigmoids/tanh descale via the
    free `scale=` affine of the ACT instruction.
  - L0's x contraction uses its true K: one DoubleRow matmul covers
    features 0..255; the tail (feature 256 + ones row carrying the fused
    input biases) is a K=2 matmul per gate, issued as row-tiled
    (tile_position) matmuls on distinct 32-row groups so the three gate
    tails plus the K=1 bhh0n bias matmul all stream concurrently.
  - L1's four bias rows are likewise a single concurrent row-tiled quad
    of K=1 matmuls instead of four serial N=512 streams.
  - Gate tail uses h' = z*prev - (z-1)*n: one fused scalar_tensor_tensor
    computes (z-1)*n, eliminating the extra sigmoid(-x) ACT op per layer.
  - The x-part matmuls for step t+1 are issued mid-step t (lookahead) so
    the PE stays busy while the serial sigmoid/tanh gate chain runs -- this
    also keeps the PE HAM clock monitor at full rate.
  - The entire MLP head collapses to out[b] = sum_t v[t]*(c_t . dnn_w) + c0
    (v = w3@w2@w1), accumulated across all 63 steps into one PSUM bank by
    M=1 matmuls against the per-step transposed state.
"""
import numpy as np
import ml_dtypes
from contextlib import ExitStack

import concourse.bass as bass
import concourse.tile as tile
from concourse import bacc, mybir
from concourse.bass_utils import run_bass_kernel_spmd

AF = mybir.ActivationFunctionType
OP = mybir.AluOpType
PM = mybir.MatmulPerfMode
F32 = mybir.dt.float32
BF16 = mybir.dt.bfloat16
FP8 = mybir.dt.float8e4
NPBF = ml_dtypes.bfloat16
NPF8 = ml_dtypes.float8_e4m3

B, T, F, H = 1024, 63, 257, 512
NCORES = 8
BC = B // NCORES          # 128 batch per core
G3 = 3 * H                # 1536
NK = H // 128             # 4 hidden chunks
NKQ = NK // 2             # 2 hidden chunk-pairs (DoubleRow)
WSCALE = 16.0             # fp8 weight pre-scale (descaled in sigmoid/tanh)
DSCALE = 4096.0           # head dnn-weight pre-scale (descaled in out copy)
U_ON_GPSIMD = True        # z*prev on the (otherwise idle) GPSIMD engine


def _dr(ap):
    """[128, 2*X] slice -> [128, 2, X] chunk-pair AP for DoubleRow."""
    return ap.rearrange("p (i b) -> p i b", i=2)


def _build_module():
    nc = bacc.Bacc("TRN2", target_bir_lowering=False, debug=False)

    xT_d = nc.dram_tensor("xT", [T, 128, 2 * BC], FP8, kind="ExternalInput").ap()
    xtl_d = nc.dram_tensor("xtl", [128, T * BC], FP8, kind="ExternalInput").ap()
    wih0_d = nc.dram_tensor("wih0", [128, 2 * G3], FP8, kind="ExternalInput").ap()
    wtl_d = nc.dram_tensor("wtl", [128, 2 * H], FP8, kind="ExternalInput").ap()
    whh0_d = nc.dram_tensor("whh0", [NKQ, 128, 2 * G3], FP8, kind="ExternalInput").ap()
    wih1_d = nc.dram_tensor("wih1", [NKQ, 128, 2 * G3], FP8, kind="ExternalInput").ap()
    whh1_d = nc.dram_tensor("whh1", [NKQ, 128, 2 * G3], FP8, kind="ExternalInput").ap()
    bw_d = nc.dram_tensor("bw", [128, 2 * H], BF16, kind="ExternalInput").ap()
    onesq_d = nc.dram_tensor("onesq", [128, BC], BF16, kind="ExternalInput").ap()
    iden_d = nc.dram_tensor("iden", [128, 128], BF16, kind="ExternalInput").ap()
    dnsc_d = nc.dram_tensor("dnsc", [NKQ, 128, 2 * 64], FP8, kind="ExternalInput").ap()
    out_d = nc.dram_tensor("out", [1, BC], F32, kind="ExternalOutput").ap()

    with tile.TileContext(nc) as tc, ExitStack() as ctx:
        wp = ctx.enter_context(tc.tile_pool(name="wp", bufs=1, space="SBUF"))
        xp = ctx.enter_context(tc.tile_pool(name="xp", bufs=4, space="SBUF"))
        sp = ctx.enter_context(tc.tile_pool(name="sp", bufs=2, space="SBUF"))
        pg = ctx.enter_context(tc.tile_pool(name="pg", bufs=3, space="PSUM"))
        pt = ctx.enter_context(tc.tile_pool(name="pt", bufs=1, space="PSUM"))
        ph = ctx.enter_context(tc.tile_pool(name="ph", bufs=1, space="PSUM"))

        # --- resident weights (chunk-pair packed for DoubleRow) ---
        wih0 = wp.tile_from(wih0_d, name="wih0")
        wtl = wp.tile_from(wtl_d, name="wtl")
        whh0 = [wp.tile_from(whh0_d[q], name=f"whh0_{q}") for q in range(NKQ)]
        wih1 = [wp.tile_from(wih1_d[q], name=f"wih1_{q}") for q in range(NKQ)]
        whh1 = [wp.tile_from(whh1_d[q], name=f"whh1_{q}") for q in range(NKQ)]
        bw = wp.tile_from(bw_d, name="bw")
        onesq = wp.tile_from(onesq_d, name="onesq")
        iden = wp.tile_from(iden_d, name="iden")
        dnsc = [wp.tile_from(dnsc_d[q], name=f"dnsc_{q}") for q in range(NKQ)]
        xtl = wp.tile_from(xtl_d, name="xtl")

        def wslice(wtile, g0, g1):
            """[128, 2*G3] pair tile -> [128, 2, g1-g0] moving operand."""
            return wtile.rearrange("p (i g) -> p i g", i=2)[:, :, g0:g1]

        head = ph.tile([1, BC], F32, name="head", tag="head", bufs=1)

        aT = None
        cT = None
        a_prev = None
        c_prev = None

        def alloc_g(nm, tag):
            pr = pg.tile([BC, H], F32, name=f"pr{nm}", tag=tag)
            pz = pg.tile([BC, H], F32, name=f"pz{nm}", tag=tag)
            phn = pg.tile([BC, H], F32, name=f"phn{nm}", tag=tag)
            pxn = pg.tile([BC, H], F32, name=f"pxn{nm}", tag=tag)
            return pr, pz, phn, pxn

        def dma_x(t):
            xt = xp.tile([128, 2 * BC], FP8, name="xt", tag="xt")
            nc.sync.dma_start(out=xt, in_=xT_d[t])
            return xt

        def l0_xpart(t, g, xt):
            """x-side matmuls for step t into L0 psum gen g=(pr,pz,phn,pxn).
            One DR matmul per gate (features 0..255) + a row-tiled quad:
            K=2 tails (feature 256 + ones->biases) for r/z/n on row groups
            0-2 and the K=1 bhh0n bias reusing group 0."""
            pr, pz, phn, pxn = g
            xq = _dr(xt)
            last = t == 0  # no hidden matmuls at t=0: close groups here
            nc.tensor.matmul(pr, xq, wslice(wih0, 0, H),
                             start=True, stop=False, perf_mode=PM.DoubleRow)
            nc.tensor.matmul(pz, xq, wslice(wih0, H, 2 * H),
                             start=True, stop=False, perf_mode=PM.DoubleRow)
            nc.tensor.matmul(pxn, xq, wslice(wih0, 2 * H, G3),
                             start=True, stop=False, perf_mode=PM.DoubleRow)
            # row groups 0/32/64 only: group 3 (base 96) hits the quadrant-3
            # XBUS HW bug. The K=1 bhh0n bias matmul reuses group 0 (row 0
            # of xtl is ones) and simply serializes after the r tail.
            tb = slice(t * BC, (t + 1) * BC)
            nc.tensor.matmul(pr, xtl[0:2, tb], wtl[0:2, 0:H],
                             start=False, stop=last, tile_position=(0, 0))
            nc.tensor.matmul(pz, xtl[32:34, tb], wtl[32:34, 0:H],
                             start=False, stop=last, tile_position=(32, 0))
            nc.tensor.matmul(pxn, xtl[64:66, tb], wtl[64:66, 0:H],
                             start=False, stop=True, tile_position=(64, 0))
            nc.tensor.matmul(phn, xtl[0:1, tb], wtl[0:1, H:2 * H],
                             start=True, stop=last, tile_position=(0, 0))

        # --- gate-chain stages, split for instruction-level interleaving ---
        def gates_sig(g, nm):
            """ACT: r = sig(pr), z = sig(pz)."""
            r = sp.tile([BC, H], BF16, name=f"r_{nm}", tag=f"r_{nm}")
            z = sp.tile([BC, H], BF16, name=f"z_{nm}", tag=f"z_{nm}")
            nc.scalar.activation(out=r, in_=g[0], func=AF.Sigmoid, scale=1.0 / WSCALE)
            nc.scalar.activation(out=z, in_=g[1], func=AF.Sigmoid, scale=1.0 / WSCALE)
            return r, z

        def gates_pre(g, r, z, prev, nm):
            """DVE: t4 = r*phn + pxn;  zc = 1-z;  GPSIMD: u = z*prev."""
            t3 = sp.tile([BC, H], BF16, name=f"t3_{nm}", tag=f"t3_{nm}")
            t4 = sp.tile([BC, H], BF16, name=f"t4_{nm}", tag=f"t4_{nm}")
            zc = sp.tile([BC, H], BF16, name=f"zc_{nm}", tag=f"zc_{nm}")
            nc.vector.tensor_tensor(out=t3, in0=r, in1=g[2], op=OP.mult)
            nc.vector.tensor_tensor(out=t4, in0=t3, in1=g[3], op=OP.add)
            nc.vector.tensor_scalar(out=zc, in0=z, scalar1=-1.0, scalar2=1.0,
                                    op0=OP.mult, op1=OP.add)
            u = None
            if prev is not None:
                u = sp.tile([BC, H], BF16, name=f"u_{nm}", tag=f"u_{nm}")
                ueng = nc.gpsimd if U_ON_GPSIMD else nc.vector
                ueng.tensor_tensor(out=u, in0=z, in1=prev, op=OP.mult)
            return t4, zc, u

        def gates_tanh(t4, nm):
            nn_t = sp.tile([BC, H], BF16, name=f"n_{nm}", tag=f"n_{nm}")
            nc.scalar.activation(out=nn_t, in_=t4, func=AF.Tanh, scale=1.0 / WSCALE)
            return nn_t

        def gates_tail(zc, nn_t, u, nm):
            """DVE: h' = (1-z)*n + u."""
            hnew = sp.tile([BC, H], BF16, name=f"h_{nm}", tag=f"h_{nm}")
            if u is None:
                nc.vector.tensor_tensor(out=hnew, in0=zc, in1=nn_t, op=OP.mult)
            else:
                t6 = sp.tile([BC, H], BF16, name=f"t6_{nm}", tag=f"t6_{nm}")
                nc.vector.tensor_tensor(out=t6, in0=zc, in1=nn_t, op=OP.mult)
                nc.vector.tensor_tensor(out=hnew, in0=t6, in1=u, op=OP.add)
            return hnew

        def tr_mms(h, nm):
            """[BC, H] SBUF bf16 -> [128, H] PSUM f32 transposed chunks.
            Regular matmul h_chunk^T @ I == transpose; unlike transpose-mode
            it counts as PE-busy for the HAM clock monitor."""
            ptr = pt.tile([128, H], F32, name=f"ptr_{nm}", tag="tr")
            for k in range(NK):
                nc.tensor.matmul(
                    ptr[:, k * 128:(k + 1) * 128],
                    h[:, k * 128:(k + 1) * 128],
                    iden,
                    start=True, stop=True,
                )
            return ptr

        def tr_copy(ptr, nm, eng):
            """PSUM f32 -> SBUF fp8, per chunk-pair (first pair ready early)."""
            hT = sp.tile([128, H], FP8, name=f"hT_{nm}", tag=f"hT_{nm}")
            for q in range(2):
                sl = slice(q * 256, (q + 1) * 256)
                if eng == "act":
                    nc.scalar.activation(out=hT[:, sl], in_=ptr[:, sl], func=AF.Copy)
                else:
                    nc.vector.tensor_copy(out=hT[:, sl], in_=ptr[:, sl])
            return hT

        def l0_hidden(g, aT):
            """L0 hidden-side DR matmuls: r, n, z order (t3 needs phn early)."""
            for gs, pdst in ((0, g[0]), (2 * H, g[2]), (H, g[1])):
                for q in range(NKQ):
                    aq = _dr(aT[:, 2 * q * 128:(2 * q + 2) * 128])
                    nc.tensor.matmul(pdst, aq, wslice(whh0[q], gs, gs + H),
                                     start=False, stop=(q == NKQ - 1),
                                     perf_mode=PM.DoubleRow)

        def l1_aside(g1, aT, close_rz):
            """L1 x-side (a-state) matmuls; closes pxn1 (and pr1/pz1 at t=0)."""
            for gs, pdst, st in ((0, g1[0], close_rz), (H, g1[1], close_rz),
                                 (2 * H, g1[3], True)):
                for q in range(NKQ):
                    aq = _dr(aT[:, 2 * q * 128:(2 * q + 2) * 128])
                    nc.tensor.matmul(pdst, aq, wslice(wih1[q], gs, gs + H),
                                     start=False, stop=(st and q == NKQ - 1),
                                     perf_mode=PM.DoubleRow)

        def l1_hidden(g1, cT):
            """L1 hidden-side DR matmuls: r, n, z; closes pr1/pz1/phn1."""
            for gs, pdst in ((0, g1[0]), (2 * H, g1[2]), (H, g1[1])):
                for q in range(NKQ):
                    cq = _dr(cT[:, 2 * q * 128:(2 * q + 2) * 128])
                    nc.tensor.matmul(pdst, cq, wslice(whh1[q], gs, gs + H),
                                     start=False, stop=(q == NKQ - 1),
                                     perf_mode=PM.DoubleRow)

        def bias_quad(t):
            """g1 alloc + row-tiled K=1 bias matmuls (concurrent trio + 1)."""
            g1 = alloc_g("1", "g1")
            nc.tensor.matmul(g1[0], onesq[0:1, :], bw[0:1, 0:H],
                             start=True, stop=False, tile_position=(0, 0))
            nc.tensor.matmul(g1[1], onesq[32:33, :], bw[32:33, 0:H],
                             start=True, stop=False, tile_position=(32, 0))
            nc.tensor.matmul(g1[2], onesq[64:65, :], bw[64:65, 0:H],
                             start=True, stop=(t == 0), tile_position=(64, 0))
            nc.tensor.matmul(g1[3], onesq[0:1, :], bw[0:1, H:2 * H],
                             start=True, stop=False, tile_position=(0, 0))
            return g1

        def head_mms(t, cT, stop=False):
            for q in range(NKQ):
                hl = dnsc[q].rearrange("p (i t) -> p i t", i=2)[:, :, t:t + 1]
                nc.tensor.matmul(head, hl,
                                 _dr(cT[:, 2 * q * 128:(2 * q + 2) * 128]),
                                 start=(t == 0 and q == 0),
                                 stop=(stop and q == NKQ - 1),
                                 perf_mode=PM.DoubleRow)

        # ---- prologue: step 0 x-side + a-gates(0), lookahead x for 1,2 ----
        xts = {t: dma_x(t) for t in range(min(3, T))}
        g0_cur = alloc_g("0", "g0")          # gen 0
        l0_xpart(0, g0_cur, xts.pop(0))
        g1_cur = bias_quad(0)                # gen 0 (bias only; no L1h)
        r0, z0 = gates_sig(g0_cur, "a")
        t40, zc0, u0 = gates_pre(g0_cur, r0, z0, None, "a")
        n0 = gates_tanh(t40, "a")
        a_state = gates_tail(zc0, n0, u0, "a")
        g0_cur = alloc_g("0", "g0")          # gen 1
        if T > 1:
            l0_xpart(1, g0_cur, xts.pop(1))

        c_state = None      # c(t-1) state: prev for c-gates AND pending tr-c
        c_new = None
        cT = None
        # Deep software pipeline, anchored on the layer-0 chain: iteration
        # tau issues tr-a(tau), L0h(tau+1), L1a(tau), tr-c(tau-1),
        # L0x(tau+2), L1h(tau), then the a-gates(tau+1) and c-gates(tau)
        # interleaved so every engine's queue matches true ready-order.
        for t in range(T):
            # E: transpose a(t); copies on ACT
            ptr_a = tr_mms(a_state, "a")
            aT = tr_copy(ptr_a, "a", "act")
            # biasq(t+1)
            if t + 1 < T:
                g1_next = bias_quad(t + 1)
            # A: L0 hidden for t+1
            if t + 1 < T:
                l0_hidden(g0_cur, aT)
                # a-sigmoids + pre (DVE) for t+1
                r0, z0 = gates_sig(g0_cur, "a")
                t40, zc0, u0 = gates_pre(g0_cur, r0, z0, a_state, "a")
            # F: L1 a-side for t
            l1_aside(g1_cur, aT, close_rz=(t == 0))
            # H: transpose c(t-1); casts placed in DVE stream here (they
            # fill the DVE bubble while tanh0 runs on ACT)
            if t > 0:
                ptr_c = tr_mms(c_new, "c")
                cT = tr_copy(ptr_c, "c", "dve")
            # tanh0(t+1)
            if t + 1 < T:
                n0 = gates_tanh(t40, "a")
            # D: x-side lookahead for t+2
            if t + 2 < T:
                if t + 3 < T:
                    xts[t + 3] = dma_x(t + 3)
                g0_cur = alloc_g("0", "g0")
                l0_xpart(t + 2, g0_cur, xts.pop(t + 2))
            # L1 hidden for t (needs cT(t-1))
            if t > 0:
                l1_hidden(g1_cur, cT)
            # a-tail: h'a(t+1)
            if t + 1 < T:
                a_next = gates_tail(zc0, n0, u0, "a")
            # head(t-1)
            if t > 0:
                head_mms(t - 1, cT)
            # c-gates(t)
            r1, z1 = gates_sig(g1_cur, "c")
            t41, zc1, u1 = gates_pre(g1_cur, r1, z1, c_state, "c")
            n1 = gates_tanh(t41, "c")
            c_next = gates_tail(zc1, n1, u1, "c")
            # rotate
            c_state = c_new = c_next
            if t + 1 < T:
                a_state = a_next
            if t + 1 < T:
                g1_cur = g1_next

        # ---- epilogue: transpose c(T-1), final head ----
        ptr_c = tr_mms(c_new, "c")
        cT = tr_copy(ptr_c, "c", "dve")
        head_mms(T - 1, cT, stop=True)
        out_sb = sp.tile([1, BC], F32, name="out_sb", tag="out_sb")
        nc.scalar.activation(out=out_sb, in_=head, func=AF.Copy, scale=1.0 / DSCALE)
        nc.sync.dma_start(out=out_d, in_=out_sb)

    # legalize sem waits (>=2 waits per matmul is a codegen error) etc.
    nc.compile()
    return nc


def _pack_pairs(wt):
    """[512, G3] (contraction-major) -> [NQ, 128, 2*G3] chunk-pair tiles:
    out[q][p, i*G3+g] = wt[(2q+i)*128 + p, g]"""
    nq = wt.shape[0] // 256
    return np.ascontiguousarray(
        wt.reshape(nq, 2, 128, -1).transpose(0, 2, 1, 3).reshape(nq, 128, -1))


def host_prep(inputs):
    f32 = np.float32
    x = np.asarray(inputs["x"], f32)
    w_ih0, w_hh0 = np.asarray(inputs["w_ih0"], f32), np.asarray(inputs["w_hh0"], f32)
    b_ih0, b_hh0 = np.asarray(inputs["b_ih0"], f32), np.asarray(inputs["b_hh0"], f32)
    w_ih1, w_hh1 = np.asarray(inputs["w_ih1"], f32), np.asarray(inputs["w_hh1"], f32)
    b_ih1, b_hh1 = np.asarray(inputs["b_ih1"], f32), np.asarray(inputs["b_hh1"], f32)
    dnn_w, dnn_b = np.asarray(inputs["dnn_w"], f32), np.asarray(inputs["dnn_b"], f32)
    w1, b1 = np.asarray(inputs["w1"], f32), np.asarray(inputs["b1"], f32)
    w2, b2 = np.asarray(inputs["w2"], f32), np.asarray(inputs["b2"], f32)
    w3, b3 = np.asarray(inputs["w3"], f32), np.asarray(inputs["b3"], f32)

    # L0 input weights: features 0..255 as one DoubleRow pair chunk; the
    # tail tile wtl carries feature 256 (row 0 of each pair) and the fused
    # biases (row 1): b_ih0+b_hh0 for r/z, b_ih0 for n; plus bhh0n at row 96.
    wihT = w_ih0.T * WSCALE                      # [F=257, G3]
    biasrow = np.concatenate([(b_ih0 + b_hh0)[:2 * H], b_ih0[2 * H:]]) * WSCALE
    wih0 = _pack_pairs(wihT[:256])[0].astype(NPF8)   # [128, 2*G3]
    # tail moving tile: row base+0 pairs with the ones row of xtl (biases),
    # row base+1 with the x256 row; cols H:2H row 0 carries bhh0n (K=1)
    wtl = np.zeros((128, 2 * H), f32)
    for gi, base in enumerate((0, 32, 64)):
        wtl[base, :H] = biasrow[gi * H:(gi + 1) * H]
        wtl[base + 1, :H] = wihT[256, gi * H:(gi + 1) * H]
    wtl[0, H:] = b_hh0[2 * H:] * WSCALE
    wtl = wtl.astype(NPF8)

    whh0 = _pack_pairs(w_hh0.T * WSCALE).astype(NPF8)
    wih1 = _pack_pairs(w_ih1.T * WSCALE).astype(NPF8)
    whh1 = _pack_pairs(w_hh1.T * WSCALE).astype(NPF8)

    # L1 bias rows for the row-tiled quad: b1r/b1z (=b_ih1+b_hh1), bhh1n, bih1n
    b1g = b_ih1 + b_hh1
    bw = np.zeros((128, 2 * H), f32)
    bw[0, :H] = b1g[:H]
    bw[32, :H] = b1g[H:2 * H]
    bw[64, :H] = b_hh1[2 * H:]
    bw[0, H:] = b_ih1[2 * H:]
    bw = (bw * WSCALE).astype(NPBF)

    v = (w3 @ w2 @ w1)[0]
    # chunk-pair packed for DoubleRow, inner dim padded 63->64 so the
    # pair-dim byte step (64) satisfies the fp8-DR step%16==0 ISA rule
    dfull = np.zeros((H, 64), f32)
    dfull[:, :T] = dnn_w[0][:, None] * v[None, :] * DSCALE
    dnsc = np.ascontiguousarray(
        dfull.reshape(NKQ, 2, 128, 64).transpose(0, 2, 1, 3)
        .reshape(NKQ, 128, 2 * 64)).astype(NPF8)
    c_all = float(v.sum() * dnn_b[0] + (w3 @ w2 @ b1)[0] + (w3 @ b2)[0] + b3[0])

    shared = dict(
        wih0=wih0, wtl=wtl, whh0=whh0, wih1=wih1, whh1=whh1, bw=bw,
        onesq=np.ones((128, BC), NPBF), iden=np.eye(128, dtype=NPBF), dnsc=dnsc)

    percore = []
    for c in range(NCORES):
        xc = x[c * BC:(c + 1) * BC]              # [BC, T, F]
        xmain = xc[:, :, :256]                   # [BC, T, 256]
        xT = (xmain.reshape(BC, T, 2, 128).transpose(1, 3, 2, 0)
              .reshape(T, 128, 2 * BC))
        xtl = np.zeros((128, T * BC), f32)
        x256 = xc[:, :, 256].T.reshape(T * BC)   # [T*BC] time-major
        for base in (0, 32, 64):
            xtl[base] = 1.0
            xtl[base + 1] = x256
        percore.append({"xT": np.ascontiguousarray(xT).astype(NPF8),
                        "xtl": xtl.astype(NPF8)})
    return shared, percore, c_all


_CACHED = {}


def _get_module():
    if "nc" not in _CACHED:
        _CACHED["nc"] = _build_module()
    return _CACHED["nc"]


def kernel(**inputs) -> np.ndarray:
    shared, percore, c_all = host_prep(inputs)
    nc = _get_module()
    in_maps = [{**shared, **percore[c]} for c in range(NCORES)]
    res = run_bass_kernel_spmd(nc, in_maps, core_ids=list(range(NCORES)))
    outs = [res.results[c]["out"].reshape(BC) for c in range(NCORES)]
    out = np.concatenate(outs).astype(np.float32) + np.float32(c_all)
    return out.reshape(B, 1)


# revision 19
# speedup vs baseline: 1.2005x; 1.0471x over previous
"""Trainium2 Bass kernel for the 2-layer GRU discriminator
(B=1024, T=63, F=257, H=512  ->  out [1024, 1]).

Strategy (pure data parallelism over batch, 8 cores x 128 batch each):
  - All weights/activations resident in SBUF; x streamed per timestep.
  - State kept as h [b=128 partitions, H free] in bf16; per-step PE
    transposes produce hT (cast to fp8) used as the matmul stationary
    operand, so gate matmuls run with the (static, SBUF-resident) weight
    matrices as the moving operand at N=512 free-dim.
  - Gate matmuls are fp8e4m3 with perf_mode=DoubleRow: K-chunk pairs are
    packed [128, 2, dim] so each matmul contracts 256 rows (2 fp8
    weights/cell), halving PE streaming time. Weights are pre-scaled by
    WSCALE=16 to stay in fp8's normal range; sigmoids/tanh descale via the
    free `scale=` affine of the ACT instruction.
  - L0's x contraction uses its true K: one DoubleRow matmul covers
    features 0..255; the tail (feature 256 + ones row carrying the fused
    input biases) is a K=2 matmul per gate, issued as row-tiled
    (tile_position) matmuls on distinct 32-row groups so the three gate
    tails plus the K=1 bhh0n bias matmul all stream concurrently.
  - L1's four bias rows are likewise a single concurrent row-tiled quad
    of K=1 matmuls instead of four serial N=512 streams.
  - Gate tail uses h' = z*prev - (z-1)*n: one fused scalar_tensor_tensor
    computes (z-1)*n, eliminating the extra sigmoid(-x) ACT op per layer.
  - The x-part matmuls for step t+1 are issued mid-step t (lookahead) so
    the PE stays busy while the serial sigmoid/tanh gate chain runs -- this
    also keeps the PE HAM clock monitor at full rate.
  - The entire MLP head collapses to out[b] = sum_t v[t]*(c_t . dnn_w) + c0
    (v = w3@w2@w1), accumulated across all 63 steps into one PSUM bank by
    M=1 matmuls against the per-step transposed state.
"""
import numpy as np
import ml_dtypes
from contextlib import ExitStack

import concourse.bass as bass
import concourse.tile as tile
from concourse import bacc, mybir
from concourse.bass_utils import run_bass_kernel_spmd

AF = mybir.ActivationFunctionType
OP = mybir.AluOpType
PM = mybir.MatmulPerfMode
F32 = mybir.dt.float32
BF16 = mybir.dt.bfloat16
FP8 = mybir.dt.float8e4
NPBF = ml_dtypes.bfloat16
NPF8 = ml_dtypes.float8_e4m3

B, T, F, H = 1024, 63, 257, 512
NCORES = 8
BC = B // NCORES          # 128 batch per core
G3 = 3 * H                # 1536
NK = H // 128             # 4 hidden chunks
NKQ = NK // 2             # 2 hidden chunk-pairs (DoubleRow)
WSCALE = 16.0             # fp8 weight pre-scale (descaled in sigmoid/tanh)
DSCALE = 4096.0           # head dnn-weight pre-scale (descaled in out copy)
U_ON_GPSIMD = True        # z*prev on the (otherwise idle) GPSIMD engine


def _dr(ap):
    """[128, 2*X] slice -> [128, 2, X] chunk-pair AP for DoubleRow."""
    return ap.rearrange("p (i b) -> p i b", i=2)


def _build_module():
    nc = bacc.Bacc("TRN2", target_bir_lowering=False, debug=False)

    xT_d = nc.dram_tensor("xT", [T, 128, 4 * BC], FP8, kind="ExternalInput").ap()
    wih0_d = nc.dram_tensor("wih0", [2, 128, 2 * G3], FP8, kind="ExternalInput").ap()
    whh0_d = nc.dram_tensor("whh0", [NKQ, 128, 2 * G3], FP8, kind="ExternalInput").ap()
    wih1_d = nc.dram_tensor("wih1", [NKQ, 128, 2 * G3], FP8, kind="ExternalInput").ap()
    whh1_d = nc.dram_tensor("whh1", [NKQ, 128, 2 * G3], FP8, kind="ExternalInput").ap()
    bw_d = nc.dram_tensor("bw", [128, 3 * H], BF16, kind="ExternalInput").ap()
    onesq_d = nc.dram_tensor("onesq", [128, BC], BF16, kind="ExternalInput").ap()
    iden_d = nc.dram_tensor("iden", [128, 128], BF16, kind="ExternalInput").ap()
    dnsc_d = nc.dram_tensor("dnsc", [NKQ, 128, 2 * 64], FP8, kind="ExternalInput").ap()
    out_d = nc.dram_tensor("out", [1, BC], F32, kind="ExternalOutput").ap()

    with tile.TileContext(nc) as tc, ExitStack() as ctx:
        wp = ctx.enter_context(tc.tile_pool(name="wp", bufs=1, space="SBUF"))
        xp = ctx.enter_context(tc.tile_pool(name="xp", bufs=4, space="SBUF"))
        sp = ctx.enter_context(tc.tile_pool(name="sp", bufs=2, space="SBUF"))
        pg = ctx.enter_context(tc.tile_pool(name="pg", bufs=3, space="PSUM"))
        pt = ctx.enter_context(tc.tile_pool(name="pt", bufs=1, space="PSUM"))
        ph = ctx.enter_context(tc.tile_pool(name="ph", bufs=1, space="PSUM"))

        # --- resident weights (chunk-pair packed for DoubleRow) ---
        wih0 = [wp.tile_from(wih0_d[q], name=f"wih0_{q}") for q in range(2)]
        whh0 = [wp.tile_from(whh0_d[q], name=f"whh0_{q}") for q in range(NKQ)]
        wih1 = [wp.tile_from(wih1_d[q], name=f"wih1_{q}") for q in range(NKQ)]
        whh1 = [wp.tile_from(whh1_d[q], name=f"whh1_{q}") for q in range(NKQ)]
        bw = wp.tile_from(bw_d, name="bw")
        onesq = wp.tile_from(onesq_d, name="onesq")
        iden = wp.tile_from(iden_d, name="iden")
        dnsc = [wp.tile_from(dnsc_d[q], name=f"dnsc_{q}") for q in range(NKQ)]

        def wslice(wtile, g0, g1):
            """[128, 2*G3] pair tile -> [128, 2, g1-g0] moving operand."""
            return wtile.rearrange("p (i g) -> p i g", i=2)[:, :, g0:g1]

        head = ph.tile([1, BC], F32, name="head", tag="head", bufs=1)

        aT = None
        cT = None
        a_prev = None
        c_prev = None

        def alloc_g(nm, tag):
            pr = pg.tile([BC, H], F32, name=f"pr{nm}", tag=tag)
            pz = pg.tile([BC, H], F32, name=f"pz{nm}", tag=tag)
            phn = pg.tile([BC, H], F32, name=f"phn{nm}", tag=tag)
            pxn = pg.tile([BC, H], F32, name=f"pxn{nm}", tag=tag)
            return pr, pz, phn, pxn

        def dma_x(t):
            xt = xp.tile([128, 4 * BC], FP8, name="xt", tag="xt")
            nc.sync.dma_start(out=xt, in_=xT_d[t])
            return xt

        def l0_xpart(t, g, xt):
            """x-side matmuls for step t into L0 psum gen g=(pr,pz,phn,pxn):
            two DoubleRow pairs per gate (x padded to 512 with a ones row at
            feature 257 carrying the fused input biases), plus a row-tiled
            K=1 matmul for the bhh0n bias."""
            pr, pz, phn, pxn = g
            last = t == 0  # no hidden matmuls at t=0: close groups here
            for q in range(2):
                xq = _dr(xt[:, 2 * q * BC:(2 * q + 2) * BC])
                nc.tensor.matmul(pr, xq, wslice(wih0[q], 0, H),
                                 start=(q == 0), stop=(last and q == 1),
                                 perf_mode=PM.DoubleRow)
                nc.tensor.matmul(pz, xq, wslice(wih0[q], H, 2 * H),
                                 start=(q == 0), stop=(last and q == 1),
                                 perf_mode=PM.DoubleRow)
                nc.tensor.matmul(pxn, xq, wslice(wih0[q], 2 * H, G3),
                                 start=(q == 0), stop=(q == 1),
                                 perf_mode=PM.DoubleRow)
            nc.tensor.matmul(phn, onesq[0:1, :], bw[0:1, 2 * H:3 * H],
                             start=True, stop=last, tile_position=(0, 0))

        # --- gate-chain stages, split for instruction-level interleaving ---
        def gates_sig(g, nm):
            """ACT: r = sig(pr), z = sig(pz)."""
            r = sp.tile([BC, H], BF16, name=f"r_{nm}", tag=f"r_{nm}")
            z = sp.tile([BC, H], BF16, name=f"z_{nm}", tag=f"z_{nm}")
            nc.scalar.activation(out=r, in_=g[0], func=AF.Sigmoid, scale=1.0 / WSCALE)
            nc.scalar.activation(out=z, in_=g[1], func=AF.Sigmoid, scale=1.0 / WSCALE)
            return r, z

        def gates_pre(g, r, z, prev, nm, split=False):
            """DVE: t4 = r*phn + pxn (optionally in H-halves for a shorter
            critical path to the low half);  zc = 1-z;  GPSIMD: u = z*prev."""
            t3 = sp.tile([BC, H], BF16, name=f"t3_{nm}", tag=f"t3_{nm}")
            t4 = sp.tile([BC, H], BF16, name=f"t4_{nm}", tag=f"t4_{nm}")
            zc = sp.tile([BC, H], BF16, name=f"zc_{nm}", tag=f"zc_{nm}")
            halves = (slice(0, H // 2), slice(H // 2, H)) if split else (slice(0, H),)
            u = None
            if prev is not None:
                u = sp.tile([BC, H], BF16, name=f"u_{nm}", tag=f"u_{nm}")
            for i, hs in enumerate(halves):
                nc.vector.tensor_tensor(out=t3[:, hs], in0=r[:, hs],
                                        in1=g[2][:, hs], op=OP.mult)
                nc.vector.tensor_tensor(out=t4[:, hs], in0=t3[:, hs],
                                        in1=g[3][:, hs], op=OP.add)
                if i == 0:
                    nc.vector.tensor_scalar(out=zc, in0=z, scalar1=-1.0,
                                            scalar2=1.0, op0=OP.mult, op1=OP.add)
                    if prev is not None:
                        ueng = nc.gpsimd if U_ON_GPSIMD else nc.vector
                        ueng.tensor_tensor(out=u, in0=z, in1=prev, op=OP.mult)
            return t4, zc, u

        def gates_tanh(t4, nm, split=False, half=None):
            if half is None:
                nn_t = sp.tile([BC, H], BF16, name=f"n_{nm}", tag=f"n_{nm}")
                hss = (slice(0, H // 2), slice(H // 2, H)) if split else (slice(0, H),)
                for hs in hss:
                    nc.scalar.activation(out=nn_t[:, hs], in_=t4[:, hs],
                                         func=AF.Tanh, scale=1.0 / WSCALE)
                return nn_t
            return None

        def gates_tail(zc, nn_t, u, nm, split=False):
            """DVE: h' = (1-z)*n + u, optionally per H-half so the low half
            of the state (and its transpose+copy) lands early."""
            hnew = sp.tile([BC, H], BF16, name=f"h_{nm}", tag=f"h_{nm}")
            hss = (slice(0, H // 2), slice(H // 2, H)) if split else (slice(0, H),)
            for hs in hss:
                if u is None:
                    nc.vector.tensor_tensor(out=hnew[:, hs], in0=zc[:, hs],
                                            in1=nn_t[:, hs], op=OP.mult)
                else:
                    t6 = sp.tile([BC, H // 2 if split else H], BF16,
                                 name=f"t6_{nm}", tag=f"t6_{nm}_{hs.start}")
                    nc.vector.tensor_tensor(out=t6, in0=zc[:, hs],
                                            in1=nn_t[:, hs], op=OP.mult)
                    nc.vector.tensor_tensor(out=hnew[:, hs], in0=t6,
                                            in1=u[:, hs], op=OP.add)
            return hnew

        def tr_mms(h, nm):
            """[BC, H] SBUF bf16 -> [128, H] PSUM f32 transposed chunks.
            Regular matmul h_chunk^T @ I == transpose; unlike transpose-mode
            it counts as PE-busy for the HAM clock monitor."""
            ptr = pt.tile([128, H], F32, name=f"ptr_{nm}", tag="tr")
            for k in range(NK):
                nc.tensor.matmul(
                    ptr[:, k * 128:(k + 1) * 128],
                    h[:, k * 128:(k + 1) * 128],
                    iden,
                    start=True, stop=True,
                )
            return ptr

        def tr_copy(ptr, nm, eng):
            """PSUM f32 -> SBUF fp8, per chunk-pair (first pair ready early)."""
            hT = sp.tile([128, H], FP8, name=f"hT_{nm}", tag=f"hT_{nm}")
            for q in range(2):
                sl = slice(q * 256, (q + 1) * 256)
                if eng == "act":
                    nc.scalar.activation(out=hT[:, sl], in_=ptr[:, sl], func=AF.Copy)
                else:
                    nc.vector.tensor_copy(out=hT[:, sl], in_=ptr[:, sl])
            return hT

        def l0_hidden(g, aT):
            """L0 hidden-side DR matmuls: r, n, z order (t3 needs phn early)."""
            for gs, pdst in ((0, g[0]), (2 * H, g[2]), (H, g[1])):
                for q in range(NKQ):
                    aq = _dr(aT[:, 2 * q * 128:(2 * q + 2) * 128])
                    nc.tensor.matmul(pdst, aq, wslice(whh0[q], gs, gs + H),
                                     start=False, stop=(q == NKQ - 1),
                                     perf_mode=PM.DoubleRow)

        def l1_aside(g1, aT, close_rz):
            """L1 x-side (a-state) matmuls; closes pxn1 (and pr1/pz1 at t=0)."""
            for gs, pdst, st in ((0, g1[0], close_rz), (H, g1[1], close_rz),
                                 (2 * H, g1[3], True)):
                for q in range(NKQ):
                    aq = _dr(aT[:, 2 * q * 128:(2 * q + 2) * 128])
                    nc.tensor.matmul(pdst, aq, wslice(wih1[q], gs, gs + H),
                                     start=False, stop=(st and q == NKQ - 1),
                                     perf_mode=PM.DoubleRow)

        def l1_hidden(g1, cT):
            """L1 hidden-side DR matmuls: r, n, z; closes pr1/pz1/phn1."""
            for gs, pdst in ((0, g1[0]), (2 * H, g1[2]), (H, g1[1])):
                for q in range(NKQ):
                    cq = _dr(cT[:, 2 * q * 128:(2 * q + 2) * 128])
                    nc.tensor.matmul(pdst, cq, wslice(whh1[q], gs, gs + H),
                                     start=False, stop=(q == NKQ - 1),
                                     perf_mode=PM.DoubleRow)

        def bias_quad(t):
            """g1 alloc + row-tiled K=1 bias matmuls (concurrent trio + 1)."""
            g1 = alloc_g("1", "g1")
            nc.tensor.matmul(g1[0], onesq[0:1, :], bw[0:1, 0:H],
                             start=True, stop=False, tile_position=(0, 0))
            nc.tensor.matmul(g1[1], onesq[32:33, :], bw[32:33, 0:H],
                             start=True, stop=False, tile_position=(32, 0))
            nc.tensor.matmul(g1[2], onesq[64:65, :], bw[64:65, 0:H],
                             start=True, stop=(t == 0), tile_position=(64, 0))
            nc.tensor.matmul(g1[3], onesq[0:1, :], bw[0:1, H:2 * H],
                             start=True, stop=False, tile_position=(0, 0))
            return g1

        def head_mms(t, cT, stop=False):
            for q in range(NKQ):
                hl = dnsc[q].rearrange("p (i t) -> p i t", i=2)[:, :, t:t + 1]
                nc.tensor.matmul(head, hl,
                                 _dr(cT[:, 2 * q * 128:(2 * q + 2) * 128]),
                                 start=(t == 0 and q == 0),
                                 stop=(stop and q == NKQ - 1),
                                 perf_mode=PM.DoubleRow)

        # ---- prologue: step 0 x-side + a-gates(0), lookahead x for 1,2 ----
        xts = {t: dma_x(t) for t in range(min(3, T))}
        g0_cur = alloc_g("0", "g0")          # gen 0
        l0_xpart(0, g0_cur, xts.pop(0))
        g1_cur = bias_quad(0)                # gen 0 (bias only; no L1h)
        r0, z0 = gates_sig(g0_cur, "a")
        t40, zc0, u0 = gates_pre(g0_cur, r0, z0, None, "a")
        n0 = gates_tanh(t40, "a")
        a_state = gates_tail(zc0, n0, u0, "a")
        g0_cur = alloc_g("0", "g0")          # gen 1
        if T > 1:
            l0_xpart(1, g0_cur, xts.pop(1))

        c_state = None      # c(t-1) state: prev for c-gates AND pending tr-c
        c_new = None
        cT = None
        # Deep software pipeline, anchored on the layer-0 chain: iteration
        # tau issues tr-a(tau), L0h(tau+1), L1a(tau), tr-c(tau-1),
        # L0x(tau+2), L1h(tau), then the a-gates(tau+1) and c-gates(tau)
        # interleaved so every engine's queue matches true ready-order.
        for t in range(T):
            # E: transpose a(t); copies on ACT
            ptr_a = tr_mms(a_state, "a")
            aT = tr_copy(ptr_a, "a", "act")
            # biasq(t+1)
            if t + 1 < T:
                g1_next = bias_quad(t + 1)
            # A: L0 hidden for t+1
            if t + 1 < T:
                l0_hidden(g0_cur, aT)
                # a-sigmoids + pre (DVE) for t+1
                r0, z0 = gates_sig(g0_cur, "a")
                t40, zc0, u0 = gates_pre(g0_cur, r0, z0, a_state, "a",
                                         split=True)
            # F: L1 a-side for t
            l1_aside(g1_cur, aT, close_rz=(t == 0))
            # H: transpose c(t-1); casts placed in DVE stream here (they
            # fill the DVE bubble while tanh0 runs on ACT)
            if t > 0:
                ptr_c = tr_mms(c_new, "c")
                cT = tr_copy(ptr_c, "c", "dve")
            # tanh0(t+1)
            if t + 1 < T:
                n0 = gates_tanh(t40, "a", split=True)
            # D: x-side lookahead for t+2
            if t + 2 < T:
                if t + 3 < T:
                    xts[t + 3] = dma_x(t + 3)
                g0_cur = alloc_g("0", "g0")
                l0_xpart(t + 2, g0_cur, xts.pop(t + 2))
            # L1 hidden for t (needs cT(t-1))
            if t > 0:
                l1_hidden(g1_cur, cT)
            # a-tail: h'a(t+1)
            if t + 1 < T:
                a_next = gates_tail(zc0, n0, u0, "a", split=True)
            # head(t-1)
            if t > 0:
                head_mms(t - 1, cT)
            # c-gates(t)
            r1, z1 = gates_sig(g1_cur, "c")
            t41, zc1, u1 = gates_pre(g1_cur, r1, z1, c_state, "c")
            n1 = gates_tanh(t41, "c")
            c_next = gates_tail(zc1, n1, u1, "c")
            # rotate
            c_state = c_new = c_next
            if t + 1 < T:
                a_state = a_next
            if t + 1 < T:
                g1_cur = g1_next

        # ---- epilogue: transpose c(T-1), final head ----
        ptr_c = tr_mms(c_new, "c")
        cT = tr_copy(ptr_c, "c", "dve")
        head_mms(T - 1, cT, stop=True)
        out_sb = sp.tile([1, BC], F32, name="out_sb", tag="out_sb")
        nc.scalar.activation(out=out_sb, in_=head, func=AF.Copy, scale=1.0 / DSCALE)
        nc.sync.dma_start(out=out_d, in_=out_sb)

    # legalize sem waits (>=2 waits per matmul is a codegen error) etc.
    nc.compile()
    return nc


def _pack_pairs(wt):
    """[512, G3] (contraction-major) -> [NQ, 128, 2*G3] chunk-pair tiles:
    out[q][p, i*G3+g] = wt[(2q+i)*128 + p, g]"""
    nq = wt.shape[0] // 256
    return np.ascontiguousarray(
        wt.reshape(nq, 2, 128, -1).transpose(0, 2, 1, 3).reshape(nq, 128, -1))


def host_prep(inputs):
    f32 = np.float32
    x = np.asarray(inputs["x"], f32)
    w_ih0, w_hh0 = np.asarray(inputs["w_ih0"], f32), np.asarray(inputs["w_hh0"], f32)
    b_ih0, b_hh0 = np.asarray(inputs["b_ih0"], f32), np.asarray(inputs["b_hh0"], f32)
    w_ih1, w_hh1 = np.asarray(inputs["w_ih1"], f32), np.asarray(inputs["w_hh1"], f32)
    b_ih1, b_hh1 = np.asarray(inputs["b_ih1"], f32), np.asarray(inputs["b_hh1"], f32)
    dnn_w, dnn_b = np.asarray(inputs["dnn_w"], f32), np.asarray(inputs["dnn_b"], f32)
    w1, b1 = np.asarray(inputs["w1"], f32), np.asarray(inputs["b1"], f32)
    w2, b2 = np.asarray(inputs["w2"], f32), np.asarray(inputs["b2"], f32)
    w3, b3 = np.asarray(inputs["w3"], f32), np.asarray(inputs["b3"], f32)

    # L0 input weights: features 0..255 as one DoubleRow pair chunk; the
    # tail tile wtl carries feature 256 (row 0 of each pair) and the fused
    # biases (row 1): b_ih0+b_hh0 for r/z, b_ih0 for n; plus bhh0n at row 96.
    wih0f = np.zeros((512, G3), f32)
    wih0f[:F] = w_ih0.T
    wih0f[F] = np.concatenate([(b_ih0 + b_hh0)[:2 * H], b_ih0[2 * H:]])
    wih0 = _pack_pairs(wih0f * WSCALE).astype(NPF8)   # [2, 128, 2*G3]
    whh0 = _pack_pairs(w_hh0.T * WSCALE).astype(NPF8)
    wih1 = _pack_pairs(w_ih1.T * WSCALE).astype(NPF8)
    whh1 = _pack_pairs(w_hh1.T * WSCALE).astype(NPF8)

    # L1 bias rows for the row-tiled quad: b1r/b1z (=b_ih1+b_hh1), bhh1n, bih1n
    b1g = b_ih1 + b_hh1
    bw = np.zeros((128, 3 * H), f32)
    bw[0, :H] = b1g[:H]
    bw[32, :H] = b1g[H:2 * H]
    bw[64, :H] = b_hh1[2 * H:]
    bw[0, H:2 * H] = b_ih1[2 * H:]
    bw[0, 2 * H:] = b_hh0[2 * H:]
    bw = (bw * WSCALE).astype(NPBF)

    v = (w3 @ w2 @ w1)[0]
    # chunk-pair packed for DoubleRow, inner dim padded 63->64 so the
    # pair-dim byte step (64) satisfies the fp8-DR step%16==0 ISA rule
    dfull = np.zeros((H, 64), f32)
    dfull[:, :T] = dnn_w[0][:, None] * v[None, :] * DSCALE
    dnsc = np.ascontiguousarray(
        dfull.reshape(NKQ, 2, 128, 64).transpose(0, 2, 1, 3)
        .reshape(NKQ, 128, 2 * 64)).astype(NPF8)
    c_all = float(v.sum() * dnn_b[0] + (w3 @ w2 @ b1)[0] + (w3 @ b2)[0] + b3[0])

    shared = dict(
        wih0=wih0, whh0=whh0, wih1=wih1, whh1=whh1, bw=bw,
        onesq=np.ones((128, BC), NPBF), iden=np.eye(128, dtype=NPBF), dnsc=dnsc)

    percore = []
    for c in range(NCORES):
        xc = x[c * BC:(c + 1) * BC]              # [BC, T, F]
        xpad = np.zeros((BC, T, 512), f32)
        xpad[:, :, :F] = xc
        xpad[:, :, F] = 1.0
        xT = (xpad.reshape(BC, T, 4, 128).transpose(1, 3, 2, 0)
              .reshape(T, 128, 4 * BC))
        percore.append({"xT": np.ascontiguousarray(xT).astype(NPF8)})
    return shared, percore, c_all


_CACHED = {}


def _get_module():
    if "nc" not in _CACHED:
        _CACHED["nc"] = _build_module()
    return _CACHED["nc"]


def kernel(**inputs) -> np.ndarray:
    shared, percore, c_all = host_prep(inputs)
    nc = _get_module()
    in_maps = [{**shared, **percore[c]} for c in range(NCORES)]
    res = run_bass_kernel_spmd(nc, in_maps, core_ids=list(range(NCORES)))
    outs = [res.results[c]["out"].reshape(BC) for c in range(NCORES)]
    out = np.concatenate(outs).astype(np.float32) + np.float32(c_all)
    return out.reshape(B, 1)


# revision 20
# speedup vs baseline: 1.2031x; 1.0022x over previous
"""Trainium2 Bass kernel for the 2-layer GRU discriminator
(B=1024, T=63, F=257, H=512  ->  out [1024, 1]).

Strategy (pure data parallelism over batch, 8 cores x 128 batch each):
  - All weights/activations resident in SBUF; x streamed per timestep.
  - State kept as h [b=128 partitions, H free] in bf16; per-step PE
    transposes produce hT (cast to fp8) used as the matmul stationary
    operand, so gate matmuls run with the (static, SBUF-resident) weight
    matrices as the moving operand at N=512 free-dim.
  - Gate matmuls are fp8e4m3 with perf_mode=DoubleRow: K-chunk pairs are
    packed [128, 2, dim] so each matmul contracts 256 rows (2 fp8
    weights/cell), halving PE streaming time. Weights are pre-scaled by
    WSCALE=16 to stay in fp8's normal range; sigmoids/tanh descale via the
    free `scale=` affine of the ACT instruction.
  - L0's x contraction uses its true K: one DoubleRow matmul covers
    features 0..255; the tail (feature 256 + ones row carrying the fused
    input biases) is a K=2 matmul per gate, issued as row-tiled
    (tile_position) matmuls on distinct 32-row groups so the three gate
    tails plus the K=1 bhh0n bias matmul all stream concurrently.
  - L1's four bias rows are likewise a single concurrent row-tiled quad
    of K=1 matmuls instead of four serial N=512 streams.
  - Gate tail uses h' = z*prev - (z-1)*n: one fused scalar_tensor_tensor
    computes (z-1)*n, eliminating the extra sigmoid(-x) ACT op per layer.
  - The x-part matmuls for step t+1 are issued mid-step t (lookahead) so
    the PE stays busy while the serial sigmoid/tanh gate chain runs -- this
    also keeps the PE HAM clock monitor at full rate.
  - The entire MLP head collapses to out[b] = sum_t v[t]*(c_t . dnn_w) + c0
    (v = w3@w2@w1), accumulated across all 63 steps into one PSUM bank by
    M=1 matmuls against the per-step transposed state.
"""
import numpy as np
import ml_dtypes
from contextlib import ExitStack

import concourse.bass as bass
import concourse.tile as tile
from concourse import bacc, mybir
from concourse.bass_utils import run_bass_kernel_spmd

AF = mybir.ActivationFunctionType
OP = mybir.AluOpType
PM = mybir.MatmulPerfMode
F32 = mybir.dt.float32
BF16 = mybir.dt.bfloat16
FP8 = mybir.dt.float8e4
NPBF = ml_dtypes.bfloat16
NPF8 = ml_dtypes.float8_e4m3

B, T, F, H = 1024, 63, 257, 512
NCORES = 8
BC = B // NCORES          # 128 batch per core
G3 = 3 * H                # 1536
NK = H // 128             # 4 hidden chunks
NKQ = NK // 2             # 2 hidden chunk-pairs (DoubleRow)
WSCALE = 16.0             # fp8 weight pre-scale (descaled in sigmoid/tanh)
DSCALE = 4096.0           # head dnn-weight pre-scale (descaled in out copy)
U_ON_GPSIMD = True        # z*prev on the (otherwise idle) GPSIMD engine


def _dr(ap):
    """[128, 2*X] slice -> [128, 2, X] chunk-pair AP for DoubleRow."""
    return ap.rearrange("p (i b) -> p i b", i=2)


def _build_module():
    nc = bacc.Bacc("TRN2", target_bir_lowering=False, debug=False)

    xT_d = nc.dram_tensor("xT", [T, 128, 4 * BC], FP8, kind="ExternalInput").ap()
    wih0_d = nc.dram_tensor("wih0", [2, 128, 2 * G3], FP8, kind="ExternalInput").ap()
    whh0_d = nc.dram_tensor("whh0", [NKQ, 128, 2 * G3], FP8, kind="ExternalInput").ap()
    wih1_d = nc.dram_tensor("wih1", [NKQ, 128, 2 * G3], FP8, kind="ExternalInput").ap()
    whh1_d = nc.dram_tensor("whh1", [NKQ, 128, 2 * G3], FP8, kind="ExternalInput").ap()
    bw_d = nc.dram_tensor("bw", [128, 3 * H], BF16, kind="ExternalInput").ap()
    onesq_d = nc.dram_tensor("onesq", [128, BC], BF16, kind="ExternalInput").ap()
    iden_d = nc.dram_tensor("iden", [128, 128], BF16, kind="ExternalInput").ap()
    dnsc_d = nc.dram_tensor("dnsc", [NKQ, 128, 2 * 64], FP8, kind="ExternalInput").ap()
    out_d = nc.dram_tensor("out", [1, BC], F32, kind="ExternalOutput").ap()

    with tile.TileContext(nc) as tc, ExitStack() as ctx:
        wp = ctx.enter_context(tc.tile_pool(name="wp", bufs=1, space="SBUF"))
        xp = ctx.enter_context(tc.tile_pool(name="xp", bufs=4, space="SBUF"))
        sp = ctx.enter_context(tc.tile_pool(name="sp", bufs=2, space="SBUF"))
        pg = ctx.enter_context(tc.tile_pool(name="pg", bufs=3, space="PSUM"))
        pt = ctx.enter_context(tc.tile_pool(name="pt", bufs=1, space="PSUM"))
        ph = ctx.enter_context(tc.tile_pool(name="ph", bufs=1, space="PSUM"))

        # --- resident weights (chunk-pair packed for DoubleRow) ---
        wih0 = [wp.tile_from(wih0_d[q], name=f"wih0_{q}") for q in range(2)]
        whh0 = [wp.tile_from(whh0_d[q], name=f"whh0_{q}") for q in range(NKQ)]
        wih1 = [wp.tile_from(wih1_d[q], name=f"wih1_{q}") for q in range(NKQ)]
        whh1 = [wp.tile_from(whh1_d[q], name=f"whh1_{q}") for q in range(NKQ)]
        bw = wp.tile_from(bw_d, name="bw")
        onesq = wp.tile_from(onesq_d, name="onesq")
        iden = wp.tile_from(iden_d, name="iden")
        dnsc = [wp.tile_from(dnsc_d[q], name=f"dnsc_{q}") for q in range(NKQ)]

        def wslice(wtile, g0, g1):
            """[128, 2*G3] pair tile -> [128, 2, g1-g0] moving operand."""
            return wtile.rearrange("p (i g) -> p i g", i=2)[:, :, g0:g1]

        head = ph.tile([1, BC], F32, name="head", tag="head", bufs=1)

        aT = None
        cT = None
        a_prev = None
        c_prev = None

        def alloc_g(nm, tag):
            pr = pg.tile([BC, H], F32, name=f"pr{nm}", tag=tag)
            pz = pg.tile([BC, H], F32, name=f"pz{nm}", tag=tag)
            phn = pg.tile([BC, H], F32, name=f"phn{nm}", tag=tag)
            pxn = pg.tile([BC, H], F32, name=f"pxn{nm}", tag=tag)
            return pr, pz, phn, pxn

        def dma_x(t):
            xt = xp.tile([128, 4 * BC], FP8, name="xt", tag="xt")
            nc.sync.dma_start(out=xt, in_=xT_d[t])
            return xt

        def l0_xpart(t, g, xt):
            """x-side matmuls for step t into L0 psum gen g=(pr,pz,phn,pxn):
            two DoubleRow pairs per gate (x padded to 512 with a ones row at
            feature 257 carrying the fused input biases), plus a row-tiled
            K=1 matmul for the bhh0n bias."""
            pr, pz, phn, pxn = g
            last = t == 0  # no hidden matmuls at t=0: close groups here
            for q in range(2):
                xq = _dr(xt[:, 2 * q * BC:(2 * q + 2) * BC])
                nc.tensor.matmul(pr, xq, wslice(wih0[q], 0, H),
                                 start=(q == 0), stop=(last and q == 1),
                                 perf_mode=PM.DoubleRow)
                nc.tensor.matmul(pz, xq, wslice(wih0[q], H, 2 * H),
                                 start=(q == 0), stop=(last and q == 1),
                                 perf_mode=PM.DoubleRow)
                nc.tensor.matmul(pxn, xq, wslice(wih0[q], 2 * H, G3),
                                 start=(q == 0), stop=(q == 1),
                                 perf_mode=PM.DoubleRow)
            nc.tensor.matmul(phn, onesq[0:1, :], bw[0:1, 2 * H:3 * H],
                             start=True, stop=last, tile_position=(0, 0))

        # --- gate-chain stages, split for instruction-level interleaving ---
        def gates_sig(g, nm):
            """ACT: r = sig(pr), z = sig(pz)."""
            r = sp.tile([BC, H], BF16, name=f"r_{nm}", tag=f"r_{nm}")
            z = sp.tile([BC, H], BF16, name=f"z_{nm}", tag=f"z_{nm}")
            nc.scalar.activation(out=r, in_=g[0], func=AF.Sigmoid, scale=1.0 / WSCALE)
            nc.scalar.activation(out=z, in_=g[1], func=AF.Sigmoid, scale=1.0 / WSCALE)
            return r, z

        def gates_pre(g, r, z, prev, nm, split=False):
            """DVE: t4 = r*phn + pxn (optionally in H-halves for a shorter
            critical path to the low half);  zc = 1-z;  GPSIMD: u = z*prev."""
            t3 = sp.tile([BC, H], BF16, name=f"t3_{nm}", tag=f"t3_{nm}")
            t4 = sp.tile([BC, H], BF16, name=f"t4_{nm}", tag=f"t4_{nm}")
            zc = sp.tile([BC, H], BF16, name=f"zc_{nm}", tag=f"zc_{nm}")
            halves = (slice(0, H // 2), slice(H // 2, H)) if split else (slice(0, H),)
            u = None
            if prev is not None:
                u = sp.tile([BC, H], BF16, name=f"u_{nm}", tag=f"u_{nm}")
            for i, hs in enumerate(halves):
                nc.vector.tensor_tensor(out=t3[:, hs], in0=r[:, hs],
                                        in1=g[2][:, hs], op=OP.mult)
                nc.vector.tensor_tensor(out=t4[:, hs], in0=t3[:, hs],
                                        in1=g[3][:, hs], op=OP.add)
                if i == 0:
                    nc.vector.tensor_scalar(out=zc, in0=z, scalar1=-1.0,
                                            scalar2=1.0, op0=OP.mult, op1=OP.add)
                    if prev is not None:
                        ueng = nc.gpsimd if U_ON_GPSIMD else nc.vector
                        ueng.tensor_tensor(out=u, in0=z, in1=prev, op=OP.mult)
            return t4, zc, u

        def gates_tanh(t4, nm, split=False, half=None):
            if half is None:
                nn_t = sp.tile([BC, H], BF16, name=f"n_{nm}", tag=f"n_{nm}")
                hss = (slice(0, H // 2), slice(H // 2, H)) if split else (slice(0, H),)
                for hs in hss:
                    nc.scalar.activation(out=nn_t[:, hs], in_=t4[:, hs],
                                         func=AF.Tanh, scale=1.0 / WSCALE)
                return nn_t
            return None

        def gates_tail(zc, nn_t, u, nm, split=False):
            """DVE: h' = (1-z)*n + u, optionally per H-half so the low half
            of the state (and its transpose+copy) lands early."""
            hnew = sp.tile([BC, H], BF16, name=f"h_{nm}", tag=f"h_{nm}")
            hss = (slice(0, H // 2), slice(H // 2, H)) if split else (slice(0, H),)
            for hs in hss:
                if u is None:
                    nc.vector.tensor_tensor(out=hnew[:, hs], in0=zc[:, hs],
                                            in1=nn_t[:, hs], op=OP.mult)
                else:
                    t6 = sp.tile([BC, H // 2 if split else H], BF16,
                                 name=f"t6_{nm}", tag=f"t6_{nm}_{hs.start}")
                    nc.vector.tensor_tensor(out=t6, in0=zc[:, hs],
                                            in1=nn_t[:, hs], op=OP.mult)
                    nc.vector.tensor_tensor(out=hnew[:, hs], in0=t6,
                                            in1=u[:, hs], op=OP.add)
            return hnew

        def tr_mms(h, nm):
            """[BC, H] SBUF bf16 -> [128, H] PSUM f32 transposed chunks.
            Regular matmul h_chunk^T @ I == transpose; unlike transpose-mode
            it counts as PE-busy for the HAM clock monitor."""
            ptr = pt.tile([128, H], F32, name=f"ptr_{nm}", tag="tr")
            for k in range(NK):
                nc.tensor.matmul(
                    ptr[:, k * 128:(k + 1) * 128],
                    h[:, k * 128:(k + 1) * 128],
                    iden,
                    start=True, stop=True,
                )
            return ptr

        def tr_copy(ptr, nm, eng):
            """PSUM f32 -> SBUF fp8, per chunk-pair (first pair ready early)."""
            hT = sp.tile([128, H], FP8, name=f"hT_{nm}", tag=f"hT_{nm}")
            for q in range(2):
                sl = slice(q * 256, (q + 1) * 256)
                if eng == "act":
                    nc.scalar.activation(out=hT[:, sl], in_=ptr[:, sl], func=AF.Copy)
                else:
                    nc.vector.tensor_copy(out=hT[:, sl], in_=ptr[:, sl])
            return hT

        def l0_hidden(g, aT):
            """L0 hidden-side DR matmuls: r, n, z order (t3 needs phn early)."""
            for gs, pdst in ((0, g[0]), (2 * H, g[2]), (H, g[1])):
                for q in range(NKQ):
                    aq = _dr(aT[:, 2 * q * 128:(2 * q + 2) * 128])
                    nc.tensor.matmul(pdst, aq, wslice(whh0[q], gs, gs + H),
                                     start=False, stop=(q == NKQ - 1),
                                     perf_mode=PM.DoubleRow)

        def l1_aside(g1, aT, close_rz):
            """L1 x-side (a-state) matmuls; closes pxn1 (and pr1/pz1 at t=0)."""
            for gs, pdst, st in ((0, g1[0], close_rz), (H, g1[1], close_rz),
                                 (2 * H, g1[3], True)):
                for q in range(NKQ):
                    aq = _dr(aT[:, 2 * q * 128:(2 * q + 2) * 128])
                    nc.tensor.matmul(pdst, aq, wslice(wih1[q], gs, gs + H),
                                     start=False, stop=(st and q == NKQ - 1),
                                     perf_mode=PM.DoubleRow)

        def l1_hidden(g1, cT):
            """L1 hidden-side DR matmuls: r, n, z; closes pr1/pz1/phn1."""
            for gs, pdst in ((0, g1[0]), (2 * H, g1[2]), (H, g1[1])):
                for q in range(NKQ):
                    cq = _dr(cT[:, 2 * q * 128:(2 * q + 2) * 128])
                    nc.tensor.matmul(pdst, cq, wslice(whh1[q], gs, gs + H),
                                     start=False, stop=(q == NKQ - 1),
                                     perf_mode=PM.DoubleRow)

        def bias_quad(t):
            """g1 alloc + row-tiled K=1 bias matmuls (concurrent trio + 1)."""
            g1 = alloc_g("1", "g1")
            nc.tensor.matmul(g1[0], onesq[0:1, :], bw[0:1, 0:H],
                             start=True, stop=False, tile_position=(0, 0))
            nc.tensor.matmul(g1[1], onesq[32:33, :], bw[32:33, 0:H],
                             start=True, stop=False, tile_position=(32, 0))
            nc.tensor.matmul(g1[2], onesq[64:65, :], bw[64:65, 0:H],
                             start=True, stop=(t == 0), tile_position=(64, 0))
            nc.tensor.matmul(g1[3], onesq[0:1, :], bw[0:1, H:2 * H],
                             start=True, stop=False, tile_position=(0, 0))
            return g1

        def head_mms(t, cT, stop=False):
            for q in range(NKQ):
                hl = dnsc[q].rearrange("p (i t) -> p i t", i=2)[:, :, t:t + 1]
                nc.tensor.matmul(head, hl,
                                 _dr(cT[:, 2 * q * 128:(2 * q + 2) * 128]),
                                 start=(t == 0 and q == 0),
                                 stop=(stop and q == NKQ - 1),
                                 perf_mode=PM.DoubleRow)

        # ---- prologue: step 0 x-side + a-gates(0), lookahead x for 1,2 ----
        xts = {t: dma_x(t) for t in range(min(3, T))}
        g0_cur = alloc_g("0", "g0")          # gen 0
        l0_xpart(0, g0_cur, xts.pop(0))
        g1_cur = bias_quad(0)                # gen 0 (bias only; no L1h)
        r0, z0 = gates_sig(g0_cur, "a")
        t40, zc0, u0 = gates_pre(g0_cur, r0, z0, None, "a")
        n0 = gates_tanh(t40, "a")
        a_state = gates_tail(zc0, n0, u0, "a")
        g0_cur = alloc_g("0", "g0")          # gen 1
        if T > 1:
            l0_xpart(1, g0_cur, xts.pop(1))

        c_state = None      # c(t-1) state: prev for c-gates AND pending tr-c
        c_new = None
        cT = None
        # Deep software pipeline, anchored on the layer-0 chain. Iteration
        # t issues (in engine-ready order): tr-a(t), tr-c(t-1), head(t-2)
        # as fillers while the aT copies run on ACT, then L0h(t+1), L1a(t),
        # L0x(t+2), L1h(t), with the a-gates(t+1) and c-gates(t) ops
        # interleaved, and the bias quad for g1(t+1) at the very end where
        # its PSUM-slot waits (freed by the c-gates just above) are short.
        cT_old = None
        for t in range(T):
            # E: transpose a(t); copies on ACT
            ptr_a = tr_mms(a_state, "a")
            aT = tr_copy(ptr_a, "a", "act")
            # H: transpose c(t-1) (computed at the end of last iter); casts
            # on DVE fill its early idle
            if t > 0:
                ptr_c = tr_mms(c_new, "c")
                cT = tr_copy(ptr_c, "c", "dve")
            # head(t-2) — PE filler while ACT does the aT copies
            if t > 1:
                head_mms(t - 2, cT_old)
            # A: L0 hidden for t+1
            if t + 1 < T:
                l0_hidden(g0_cur, aT)
                r0, z0 = gates_sig(g0_cur, "a")
                t40, zc0, u0 = gates_pre(g0_cur, r0, z0, a_state, "a",
                                         split=True)
            # F: L1 a-side for t
            l1_aside(g1_cur, aT, close_rz=(t == 0))
            # tanh0(t+1)
            if t + 1 < T:
                n0 = gates_tanh(t40, "a", split=True)
            # D: x-side lookahead for t+2
            if t + 2 < T:
                if t + 3 < T:
                    xts[t + 3] = dma_x(t + 3)
                g0_cur = alloc_g("0", "g0")
                l0_xpart(t + 2, g0_cur, xts.pop(t + 2))
            # L1 hidden for t (needs cT(t-1))
            if t > 0:
                l1_hidden(g1_cur, cT)
            # a-tail: h'a(t+1)
            if t + 1 < T:
                a_next = gates_tail(zc0, n0, u0, "a", split=True)
            # c-gates(t)
            r1, z1 = gates_sig(g1_cur, "c")
            t41, zc1, u1 = gates_pre(g1_cur, r1, z1, c_state, "c")
            n1 = gates_tanh(t41, "c")
            c_next = gates_tail(zc1, n1, u1, "c")
            # biasq(t+1): its g1-ring slots were just freed by the c-gates
            if t + 1 < T:
                g1_next = bias_quad(t + 1)
            # rotate
            c_state = c_new = c_next
            cT_old = cT if t > 0 else None
            if t + 1 < T:
                a_state = a_next
                g1_cur = g1_next

        # ---- epilogue: transpose c(T-1), remaining heads ----
        ptr_c = tr_mms(c_new, "c")
        cT = tr_copy(ptr_c, "c", "dve")
        head_mms(T - 2, cT_old)
        head_mms(T - 1, cT, stop=True)
        out_sb = sp.tile([1, BC], F32, name="out_sb", tag="out_sb")
        nc.scalar.activation(out=out_sb, in_=head, func=AF.Copy, scale=1.0 / DSCALE)
        nc.sync.dma_start(out=out_d, in_=out_sb)

    # legalize sem waits (>=2 waits per matmul is a codegen error) etc.
    nc.compile()
    return nc


def _pack_pairs(wt):
    """[512, G3] (contraction-major) -> [NQ, 128, 2*G3] chunk-pair tiles:
    out[q][p, i*G3+g] = wt[(2q+i)*128 + p, g]"""
    nq = wt.shape[0] // 256
    return np.ascontiguousarray(
        wt.reshape(nq, 2, 128, -1).transpose(0, 2, 1, 3).reshape(nq, 128, -1))


def host_prep(inputs):
    f32 = np.float32
    x = np.asarray(inputs["x"], f32)
    w_ih0, w_hh0 = np.asarray(inputs["w_ih0"], f32), np.asarray(inputs["w_hh0"], f32)
    b_ih0, b_hh0 = np.asarray(inputs["b_ih0"], f32), np.asarray(inputs["b_hh0"], f32)
    w_ih1, w_hh1 = np.asarray(inputs["w_ih1"], f32), np.asarray(inputs["w_hh1"], f32)
    b_ih1, b_hh1 = np.asarray(inputs["b_ih1"], f32), np.asarray(inputs["b_hh1"], f32)
    dnn_w, dnn_b = np.asarray(inputs["dnn_w"], f32), np.asarray(inputs["dnn_b"], f32)
    w1, b1 = np.asarray(inputs["w1"], f32), np.asarray(inputs["b1"], f32)
    w2, b2 = np.asarray(inputs["w2"], f32), np.asarray(inputs["b2"], f32)
    w3, b3 = np.asarray(inputs["w3"], f32), np.asarray(inputs["b3"], f32)

    # L0 input weights: features 0..255 as one DoubleRow pair chunk; the
    # tail tile wtl carries feature 256 (row 0 of each pair) and the fused
    # biases (row 1): b_ih0+b_hh0 for r/z, b_ih0 for n; plus bhh0n at row 96.
    wih0f = np.zeros((512, G3), f32)
    wih0f[:F] = w_ih0.T
    wih0f[F] = np.concatenate([(b_ih0 + b_hh0)[:2 * H], b_ih0[2 * H:]])
    wih0 = _pack_pairs(wih0f * WSCALE).astype(NPF8)   # [2, 128, 2*G3]
    whh0 = _pack_pairs(w_hh0.T * WSCALE).astype(NPF8)
    wih1 = _pack_pairs(w_ih1.T * WSCALE).astype(NPF8)
    whh1 = _pack_pairs(w_hh1.T * WSCALE).astype(NPF8)

    # L1 bias rows for the row-tiled quad: b1r/b1z (=b_ih1+b_hh1), bhh1n, bih1n
    b1g = b_ih1 + b_hh1
    bw = np.zeros((128, 3 * H), f32)
    bw[0, :H] = b1g[:H]
    bw[32, :H] = b1g[H:2 * H]
    bw[64, :H] = b_hh1[2 * H:]
    bw[0, H:2 * H] = b_ih1[2 * H:]
    bw[0, 2 * H:] = b_hh0[2 * H:]
    bw = (bw * WSCALE).astype(NPBF)

    v = (w3 @ w2 @ w1)[0]
    # chunk-pair packed for DoubleRow, inner dim padded 63->64 so the
    # pair-dim byte step (64) satisfies the fp8-DR step%16==0 ISA rule
    dfull = np.zeros((H, 64), f32)
    dfull[:, :T] = dnn_w[0][:, None] * v[None, :] * DSCALE
    dnsc = np.ascontiguousarray(
        dfull.reshape(NKQ, 2, 128, 64).transpose(0, 2, 1, 3)
        .reshape(NKQ, 128, 2 * 64)).astype(NPF8)
    c_all = float(v.sum() * dnn_b[0] + (w3 @ w2 @ b1)[0] + (w3 @ b2)[0] + b3[0])

    shared = dict(
        wih0=wih0, whh0=whh0, wih1=wih1, whh1=whh1, bw=bw,
        onesq=np.ones((128, BC), NPBF), iden=np.eye(128, dtype=NPBF), dnsc=dnsc)

    percore = []
    for c in range(NCORES):
        xc = x[c * BC:(c + 1) * BC]              # [BC, T, F]
        xpad = np.zeros((BC, T, 512), f32)
        xpad[:, :, :F] = xc
        xpad[:, :, F] = 1.0
        xT = (xpad.reshape(BC, T, 4, 128).transpose(1, 3, 2, 0)
              .reshape(T, 128, 4 * BC))
        percore.append({"xT": np.ascontiguousarray(xT).astype(NPF8)})
    return shared, percore, c_all


_CACHED = {}


def _get_module():
    if "nc" not in _CACHED:
        _CACHED["nc"] = _build_module()
    return _CACHED["nc"]


def kernel(**inputs) -> np.ndarray:
    shared, percore, c_all = host_prep(inputs)
    nc = _get_module()
    in_maps = [{**shared, **percore[c]} for c in range(NCORES)]
    res = run_bass_kernel_spmd(nc, in_maps, core_ids=list(range(NCORES)))
    outs = [res.results[c]["out"].reshape(BC) for c in range(NCORES)]
    out = np.concatenate(outs).astype(np.float32) + np.float32(c_all)
    return out.reshape(B, 1)


# revision 21
# speedup vs baseline: 1.2058x; 1.0022x over previous
"""Trainium2 Bass kernel for the 2-layer GRU discriminator
(B=1024, T=63, F=257, H=512  ->  out [1024, 1]).

Strategy (pure data parallelism over batch, 8 cores x 128 batch each):
  - All weights/activations resident in SBUF; x streamed per timestep.
  - State kept as h [b=128 partitions, H free] in bf16; per-step PE
    transposes produce hT (cast to fp8) used as the matmul stationary
    operand, so gate matmuls run with the (static, SBUF-resident) weight
    matrices as the moving operand at N=512 free-dim.
  - Gate matmuls are fp8e4m3 with perf_mode=DoubleRow: K-chunk pairs are
    packed [128, 2, dim] so each matmul contracts 256 rows (2 fp8
    weights/cell), halving PE streaming time. Weights are pre-scaled by
    WSCALE=16 to stay in fp8's normal range; sigmoids/tanh descale via the
    free `scale=` affine of the ACT instruction.
  - L0's x contraction uses its true K: one DoubleRow matmul covers
    features 0..255; the tail (feature 256 + ones row carrying the fused
    input biases) is a K=2 matmul per gate, issued as row-tiled
    (tile_position) matmuls on distinct 32-row groups so the three gate
    tails plus the K=1 bhh0n bias matmul all stream concurrently.
  - L1's four bias rows are likewise a single concurrent row-tiled quad
    of K=1 matmuls instead of four serial N=512 streams.
  - Gate tail uses h' = z*prev - (z-1)*n: one fused scalar_tensor_tensor
    computes (z-1)*n, eliminating the extra sigmoid(-x) ACT op per layer.
  - The x-part matmuls for step t+1 are issued mid-step t (lookahead) so
    the PE stays busy while the serial sigmoid/tanh gate chain runs -- this
    also keeps the PE HAM clock monitor at full rate.
  - The entire MLP head collapses to out[b] = sum_t v[t]*(c_t . dnn_w) + c0
    (v = w3@w2@w1), accumulated across all 63 steps into one PSUM bank by
    M=1 matmuls against the per-step transposed state.
"""
import numpy as np
import ml_dtypes
from contextlib import ExitStack

import concourse.bass as bass
import concourse.tile as tile
from concourse import bacc, mybir
from concourse.bass_utils import run_bass_kernel_spmd

AF = mybir.ActivationFunctionType
OP = mybir.AluOpType
PM = mybir.MatmulPerfMode
F32 = mybir.dt.float32
BF16 = mybir.dt.bfloat16
FP8 = mybir.dt.float8e4
NPBF = ml_dtypes.bfloat16
NPF8 = ml_dtypes.float8_e4m3

B, T, F, H = 1024, 63, 257, 512
NCORES = 8
BC = B // NCORES          # 128 batch per core
G3 = 3 * H                # 1536
NK = H // 128             # 4 hidden chunks
NKQ = NK // 2             # 2 hidden chunk-pairs (DoubleRow)
WSCALE = 16.0             # fp8 weight pre-scale (descaled in sigmoid/tanh)
DSCALE = 4096.0           # head dnn-weight pre-scale (descaled in out copy)
U_ON_GPSIMD = True        # z*prev on the (otherwise idle) GPSIMD engine


def _dr(ap):
    """[128, 2*X] slice -> [128, 2, X] chunk-pair AP for DoubleRow."""
    return ap.rearrange("p (i b) -> p i b", i=2)


def _build_module():
    nc = bacc.Bacc("TRN2", target_bir_lowering=False, debug=False)

    xT_d = nc.dram_tensor("xT", [T, 128, 4 * BC], FP8, kind="ExternalInput").ap()
    wih0_d = nc.dram_tensor("wih0", [2, 128, 2 * G3], FP8, kind="ExternalInput").ap()
    whh0_d = nc.dram_tensor("whh0", [NKQ, 128, 2 * G3], FP8, kind="ExternalInput").ap()
    wih1_d = nc.dram_tensor("wih1", [NKQ, 128, 2 * G3], FP8, kind="ExternalInput").ap()
    whh1_d = nc.dram_tensor("whh1", [NKQ, 128, 2 * G3], FP8, kind="ExternalInput").ap()
    bw_d = nc.dram_tensor("bw", [128, 3 * H], BF16, kind="ExternalInput").ap()
    onesq_d = nc.dram_tensor("onesq", [128, BC], BF16, kind="ExternalInput").ap()
    iden_d = nc.dram_tensor("iden", [128, 128], BF16, kind="ExternalInput").ap()
    dnsc_d = nc.dram_tensor("dnsc", [NKQ, 128, 2 * 64], FP8, kind="ExternalInput").ap()
    out_d = nc.dram_tensor("out", [1, BC], F32, kind="ExternalOutput").ap()

    with tile.TileContext(nc) as tc, ExitStack() as ctx:
        wp = ctx.enter_context(tc.tile_pool(name="wp", bufs=1, space="SBUF"))
        xp = ctx.enter_context(tc.tile_pool(name="xp", bufs=4, space="SBUF"))
        sp = ctx.enter_context(tc.tile_pool(name="sp", bufs=2, space="SBUF"))
        pg = ctx.enter_context(tc.tile_pool(name="pg", bufs=3, space="PSUM"))
        pt = ctx.enter_context(tc.tile_pool(name="pt", bufs=1, space="PSUM"))
        ph = ctx.enter_context(tc.tile_pool(name="ph", bufs=1, space="PSUM"))

        # --- resident weights (chunk-pair packed for DoubleRow) ---
        wih0 = [wp.tile_from(wih0_d[q], name=f"wih0_{q}") for q in range(2)]
        whh0 = [wp.tile_from(whh0_d[q], name=f"whh0_{q}") for q in range(NKQ)]
        wih1 = [wp.tile_from(wih1_d[q], name=f"wih1_{q}") for q in range(NKQ)]
        whh1 = [wp.tile_from(whh1_d[q], name=f"whh1_{q}") for q in range(NKQ)]
        bw = wp.tile_from(bw_d, name="bw")
        onesq = wp.tile_from(onesq_d, name="onesq")
        iden = wp.tile_from(iden_d, name="iden")
        dnsc = [wp.tile_from(dnsc_d[q], name=f"dnsc_{q}") for q in range(NKQ)]

        def wslice(wtile, g0, g1):
            """[128, 2*G3] pair tile -> [128, 2, g1-g0] moving operand."""
            return wtile.rearrange("p (i g) -> p i g", i=2)[:, :, g0:g1]

        head = ph.tile([1, BC], F32, name="head", tag="head", bufs=1)

        aT = None
        cT = None
        a_prev = None
        c_prev = None

        def alloc_g(nm, tag):
            pr = pg.tile([BC, H], F32, name=f"pr{nm}", tag=tag)
            pz = pg.tile([BC, H], F32, name=f"pz{nm}", tag=tag)
            phn = pg.tile([BC, H], F32, name=f"phn{nm}", tag=tag)
            pxn = pg.tile([BC, H], F32, name=f"pxn{nm}", tag=tag)
            return pr, pz, phn, pxn

        def dma_x(t):
            xt = xp.tile([128, 4 * BC], FP8, name="xt", tag="xt")
            nc.sync.dma_start(out=xt, in_=xT_d[t])
            return xt

        def l0_xpart(t, g, xt):
            """x-side matmuls for step t into L0 psum gen g=(pr,pz,phn,pxn):
            two DoubleRow pairs per gate (x padded to 512 with a ones row at
            feature 257 carrying the fused input biases), plus a row-tiled
            K=1 matmul for the bhh0n bias."""
            pr, pz, phn, pxn = g
            last = t == 0  # no hidden matmuls at t=0: close groups here
            for q in range(2):
                xq = _dr(xt[:, 2 * q * BC:(2 * q + 2) * BC])
                nc.tensor.matmul(pr, xq, wslice(wih0[q], 0, H),
                                 start=(q == 0), stop=(last and q == 1),
                                 perf_mode=PM.DoubleRow)
                nc.tensor.matmul(pz, xq, wslice(wih0[q], H, 2 * H),
                                 start=(q == 0), stop=(last and q == 1),
                                 perf_mode=PM.DoubleRow)
                nc.tensor.matmul(pxn, xq, wslice(wih0[q], 2 * H, G3),
                                 start=(q == 0), stop=(q == 1),
                                 perf_mode=PM.DoubleRow)
            nc.tensor.matmul(phn, onesq[0:1, :], bw[0:1, 2 * H:3 * H],
                             start=True, stop=last, tile_position=(0, 0))

        # --- gate-chain stages, split for instruction-level interleaving ---
        def gates_sig(g, nm):
            """ACT: r = sig(pr), z = sig(pz)."""
            r = sp.tile([BC, H], BF16, name=f"r_{nm}", tag=f"r_{nm}")
            z = sp.tile([BC, H], BF16, name=f"z_{nm}", tag=f"z_{nm}")
            nc.scalar.activation(out=r, in_=g[0], func=AF.Sigmoid, scale=1.0 / WSCALE)
            nc.scalar.activation(out=z, in_=g[1], func=AF.Sigmoid, scale=1.0 / WSCALE)
            return r, z

        def gates_pre(g, r, z, prev, nm, split=False):
            """DVE: t4 = r*phn + pxn (optionally in H-halves for a shorter
            critical path to the low half);  zc = 1-z;  GPSIMD: u = z*prev."""
            t3 = sp.tile([BC, H], BF16, name=f"t3_{nm}", tag=f"t3_{nm}")
            t4 = sp.tile([BC, H], BF16, name=f"t4_{nm}", tag=f"t4_{nm}")
            zc = sp.tile([BC, H], BF16, name=f"zc_{nm}", tag=f"zc_{nm}")
            halves = (slice(0, H // 2), slice(H // 2, H)) if split else (slice(0, H),)
            u = None
            if prev is not None:
                u = sp.tile([BC, H], BF16, name=f"u_{nm}", tag=f"u_{nm}")
            for i, hs in enumerate(halves):
                nc.vector.tensor_tensor(out=t3[:, hs], in0=r[:, hs],
                                        in1=g[2][:, hs], op=OP.mult)
                nc.vector.tensor_tensor(out=t4[:, hs], in0=t3[:, hs],
                                        in1=g[3][:, hs], op=OP.add)
                if i == 0:
                    nc.vector.tensor_scalar(out=zc, in0=z, scalar1=-1.0,
                                            scalar2=1.0, op0=OP.mult, op1=OP.add)
                    if prev is not None:
                        ueng = nc.gpsimd if U_ON_GPSIMD else nc.vector
                        ueng.tensor_tensor(out=u, in0=z, in1=prev, op=OP.mult)
            return t4, zc, u

        def gates_tanh(t4, nm, split=False, half=None):
            if half is None:
                nn_t = sp.tile([BC, H], BF16, name=f"n_{nm}", tag=f"n_{nm}")
                hss = (slice(0, H // 2), slice(H // 2, H)) if split else (slice(0, H),)
                for hs in hss:
                    nc.scalar.activation(out=nn_t[:, hs], in_=t4[:, hs],
                                         func=AF.Tanh, scale=1.0 / WSCALE)
                return nn_t
            return None

        def gates_tail(zc, nn_t, u, nm, split=False):
            """DVE: h' = (1-z)*n + u, optionally per H-half so the low half
            of the state (and its transpose+copy) lands early."""
            hnew = sp.tile([BC, H], BF16, name=f"h_{nm}", tag=f"h_{nm}")
            hss = (slice(0, H // 2), slice(H // 2, H)) if split else (slice(0, H),)
            for hs in hss:
                if u is None:
                    nc.vector.tensor_tensor(out=hnew[:, hs], in0=zc[:, hs],
                                            in1=nn_t[:, hs], op=OP.mult)
                else:
                    t6 = sp.tile([BC, H // 2 if split else H], BF16,
                                 name=f"t6_{nm}", tag=f"t6_{nm}_{hs.start}")
                    nc.vector.tensor_tensor(out=t6, in0=zc[:, hs],
                                            in1=nn_t[:, hs], op=OP.mult)
                    nc.vector.tensor_tensor(out=hnew[:, hs], in0=t6,
                                            in1=u[:, hs], op=OP.add)
            return hnew

        def tr_mms(h, nm):
            """[BC, H] SBUF bf16 -> [128, H] PSUM f32 transposed chunks.
            Regular matmul h_chunk^T @ I == transpose; unlike transpose-mode
            it counts as PE-busy for the HAM clock monitor."""
            ptr = pt.tile([128, H], F32, name=f"ptr_{nm}", tag="tr")
            for k in range(NK):
                nc.tensor.matmul(
                    ptr[:, k * 128:(k + 1) * 128],
                    h[:, k * 128:(k + 1) * 128],
                    iden,
                    start=True, stop=True,
                )
            return ptr

        def tr_copy(ptr, nm, eng):
            """PSUM f32 -> SBUF fp8, per chunk-pair (first pair ready early)."""
            hT = sp.tile([128, H], FP8, name=f"hT_{nm}", tag=f"hT_{nm}")
            for q in range(2):
                sl = slice(q * 256, (q + 1) * 256)
                if eng == "act":
                    nc.scalar.activation(out=hT[:, sl], in_=ptr[:, sl], func=AF.Copy)
                else:
                    nc.vector.tensor_copy(out=hT[:, sl], in_=ptr[:, sl])
            return hT

        def l0_hidden(g, aT):
            """L0 hidden-side DR matmuls: r, n, z order (t3 needs phn early)."""
            for gs, pdst in ((0, g[0]), (2 * H, g[2]), (H, g[1])):
                for q in range(NKQ):
                    aq = _dr(aT[:, 2 * q * 128:(2 * q + 2) * 128])
                    nc.tensor.matmul(pdst, aq, wslice(whh0[q], gs, gs + H),
                                     start=False, stop=(q == NKQ - 1),
                                     perf_mode=PM.DoubleRow)

        def l1_aside(g1, aT, close_rz):
            """L1 x-side (a-state) matmuls; closes pxn1 (and pr1/pz1 at t=0)."""
            for gs, pdst, st in ((0, g1[0], close_rz), (H, g1[1], close_rz),
                                 (2 * H, g1[3], True)):
                for q in range(NKQ):
                    aq = _dr(aT[:, 2 * q * 128:(2 * q + 2) * 128])
                    nc.tensor.matmul(pdst, aq, wslice(wih1[q], gs, gs + H),
                                     start=False, stop=(st and q == NKQ - 1),
                                     perf_mode=PM.DoubleRow)

        def l1_hidden(g1, cT):
            """L1 hidden-side DR matmuls: r, n, z; closes pr1/pz1/phn1."""
            for gs, pdst in ((0, g1[0]), (2 * H, g1[2]), (H, g1[1])):
                for q in range(NKQ):
                    cq = _dr(cT[:, 2 * q * 128:(2 * q + 2) * 128])
                    nc.tensor.matmul(pdst, cq, wslice(whh1[q], gs, gs + H),
                                     start=False, stop=(q == NKQ - 1),
                                     perf_mode=PM.DoubleRow)

        def bias_quad(t):
            """g1 alloc + row-tiled K=1 bias matmuls (concurrent trio + 1)."""
            g1 = alloc_g("1", "g1")
            nc.tensor.matmul(g1[0], onesq[0:1, :], bw[0:1, 0:H],
                             start=True, stop=False, tile_position=(0, 0))
            nc.tensor.matmul(g1[1], onesq[32:33, :], bw[32:33, 0:H],
                             start=True, stop=False, tile_position=(32, 0))
            nc.tensor.matmul(g1[2], onesq[64:65, :], bw[64:65, 0:H],
                             start=True, stop=(t == 0), tile_position=(64, 0))
            nc.tensor.matmul(g1[3], onesq[0:1, :], bw[0:1, H:2 * H],
                             start=True, stop=False, tile_position=(0, 0))
            return g1

        def head_mms(t, cT, stop=False):
            for q in range(NKQ):
                hl = dnsc[q].rearrange("p (i t) -> p i t", i=2)[:, :, t:t + 1]
                nc.tensor.matmul(head, hl,
                                 _dr(cT[:, 2 * q * 128:(2 * q + 2) * 128]),
                                 start=(t == 0 and q == 0),
                                 stop=(stop and q == NKQ - 1),
                                 perf_mode=PM.DoubleRow)

        # ---- prologue: step 0 x-side + a-gates(0), lookahead x for 1,2 ----
        xts = {t: dma_x(t) for t in range(min(3, T))}
        g0_cur = alloc_g("0", "g0")          # gen 0
        l0_xpart(0, g0_cur, xts.pop(0))
        g1_cur = bias_quad(0)                # gen 0 (bias only; no L1h)
        r0, z0 = gates_sig(g0_cur, "a")
        t40, zc0, u0 = gates_pre(g0_cur, r0, z0, None, "a")
        n0 = gates_tanh(t40, "a")
        a_state = gates_tail(zc0, n0, u0, "a")
        ptr_a = tr_mms(a_state, "a")
        aT = tr_copy(ptr_a, "a", "act")
        g0_cur = alloc_g("0", "g0")          # gen 1
        if T > 1:
            l0_xpart(1, g0_cur, xts.pop(1))

        c_state = None      # c(t-1) state: prev for c-gates AND pending tr-c
        c_new = None
        cT = None
        # Deep software pipeline, anchored on the layer-0 chain. The
        # transpose+copy of a(t+1) happens at the END of iteration t (right
        # after h'a(t+1) is produced), so iteration t+1's L0h matmuls start
        # with zero wait -- the PE never idles at the iteration boundary and
        # the HAM clock monitor stays at full rate.
        cT_old = None
        for t in range(T):
            # A: L0 hidden for t+1 (aT(t) transposed+copied last iter)
            if t + 1 < T:
                l0_hidden(g0_cur, aT)
                r0, z0 = gates_sig(g0_cur, "a")
                t40, zc0, u0 = gates_pre(g0_cur, r0, z0, a_state, "a",
                                         split=True)
            # F: L1 a-side for t
            l1_aside(g1_cur, aT, close_rz=(t == 0))
            # H: transpose c(t-1) (from the end of last iter); casts on DVE
            if t > 0:
                ptr_c = tr_mms(c_new, "c")
                cT = tr_copy(ptr_c, "c", "dve")
            # head(t-2) — small PE filler
            if t > 1:
                head_mms(t - 2, cT_old)
            # tanh0(t+1) in halves
            if t + 1 < T:
                n0 = gates_tanh(t40, "a", split=True)
            # D: x-side lookahead for t+2
            if t + 2 < T:
                if t + 3 < T:
                    xts[t + 3] = dma_x(t + 3)
                g0_cur = alloc_g("0", "g0")
                l0_xpart(t + 2, g0_cur, xts.pop(t + 2))
            # a-tail: h'a(t+1), low half first
            if t + 1 < T:
                a_next = gates_tail(zc0, n0, u0, "a", split=True)
                # E: transpose + copy a(t+1) now — before L1h in the PE
                # queue and before the c-sigmoids in the ACT queue
                ptr_a = tr_mms(a_next, "a")
                aT_next = tr_copy(ptr_a, "a", "act")
            # L1 hidden for t (needs cT(t-1))
            if t > 0:
                l1_hidden(g1_cur, cT)
            # c-gates(t)
            r1, z1 = gates_sig(g1_cur, "c")
            t41, zc1, u1 = gates_pre(g1_cur, r1, z1, c_state, "c")
            n1 = gates_tanh(t41, "c")
            c_next = gates_tail(zc1, n1, u1, "c")
            # biasq(t+1): its g1-ring slots were just freed by the c-gates
            if t + 1 < T:
                g1_next = bias_quad(t + 1)
            # rotate
            c_state = c_new = c_next
            cT_old = cT if t > 0 else None
            if t + 1 < T:
                a_state = a_next
                aT = aT_next
                g1_cur = g1_next

        # ---- epilogue: transpose c(T-1), remaining heads ----
        ptr_c = tr_mms(c_new, "c")
        cT = tr_copy(ptr_c, "c", "dve")
        head_mms(T - 2, cT_old)
        head_mms(T - 1, cT, stop=True)
        out_sb = sp.tile([1, BC], F32, name="out_sb", tag="out_sb")
        nc.scalar.activation(out=out_sb, in_=head, func=AF.Copy, scale=1.0 / DSCALE)
        nc.sync.dma_start(out=out_d, in_=out_sb)

    # legalize sem waits (>=2 waits per matmul is a codegen error) etc.
    nc.compile()
    return nc


def _pack_pairs(wt):
    """[512, G3] (contraction-major) -> [NQ, 128, 2*G3] chunk-pair tiles:
    out[q][p, i*G3+g] = wt[(2q+i)*128 + p, g]"""
    nq = wt.shape[0] // 256
    return np.ascontiguousarray(
        wt.reshape(nq, 2, 128, -1).transpose(0, 2, 1, 3).reshape(nq, 128, -1))


def host_prep(inputs):
    f32 = np.float32
    x = np.asarray(inputs["x"], f32)
    w_ih0, w_hh0 = np.asarray(inputs["w_ih0"], f32), np.asarray(inputs["w_hh0"], f32)
    b_ih0, b_hh0 = np.asarray(inputs["b_ih0"], f32), np.asarray(inputs["b_hh0"], f32)
    w_ih1, w_hh1 = np.asarray(inputs["w_ih1"], f32), np.asarray(inputs["w_hh1"], f32)
    b_ih1, b_hh1 = np.asarray(inputs["b_ih1"], f32), np.asarray(inputs["b_hh1"], f32)
    dnn_w, dnn_b = np.asarray(inputs["dnn_w"], f32), np.asarray(inputs["dnn_b"], f32)
    w1, b1 = np.asarray(inputs["w1"], f32), np.asarray(inputs["b1"], f32)
    w2, b2 = np.asarray(inputs["w2"], f32), np.asarray(inputs["b2"], f32)
    w3, b3 = np.asarray(inputs["w3"], f32), np.asarray(inputs["b3"], f32)

    # L0 input weights: features 0..255 as one DoubleRow pair chunk; the
    # tail tile wtl carries feature 256 (row 0 of each pair) and the fused
    # biases (row 1): b_ih0+b_hh0 for r/z, b_ih0 for n; plus bhh0n at row 96.
    wih0f = np.zeros((512, G3), f32)
    wih0f[:F] = w_ih0.T
    wih0f[F] = np.concatenate([(b_ih0 + b_hh0)[:2 * H], b_ih0[2 * H:]])
    wih0 = _pack_pairs(wih0f * WSCALE).astype(NPF8)   # [2, 128, 2*G3]
    whh0 = _pack_pairs(w_hh0.T * WSCALE).astype(NPF8)
    wih1 = _pack_pairs(w_ih1.T * WSCALE).astype(NPF8)
    whh1 = _pack_pairs(w_hh1.T * WSCALE).astype(NPF8)

    # L1 bias rows for the row-tiled quad: b1r/b1z (=b_ih1+b_hh1), bhh1n, bih1n
    b1g = b_ih1 + b_hh1
    bw = np.zeros((128, 3 * H), f32)
    bw[0, :H] = b1g[:H]
    bw[32, :H] = b1g[H:2 * H]
    bw[64, :H] = b_hh1[2 * H:]
    bw[0, H:2 * H] = b_ih1[2 * H:]
    bw[0, 2 * H:] = b_hh0[2 * H:]
    bw = (bw * WSCALE).astype(NPBF)

    v = (w3 @ w2 @ w1)[0]
    # chunk-pair packed for DoubleRow, inner dim padded 63->64 so the
    # pair-dim byte step (64) satisfies the fp8-DR step%16==0 ISA rule
    dfull = np.zeros((H, 64), f32)
    dfull[:, :T] = dnn_w[0][:, None] * v[None, :] * DSCALE
    dnsc = np.ascontiguousarray(
        dfull.reshape(NKQ, 2, 128, 64).transpose(0, 2, 1, 3)
        .reshape(NKQ, 128, 2 * 64)).astype(NPF8)
    c_all = float(v.sum() * dnn_b[0] + (w3 @ w2 @ b1)[0] + (w3 @ b2)[0] + b3[0])

    shared = dict(
        wih0=wih0, whh0=whh0, wih1=wih1, whh1=whh1, bw=bw,
        onesq=np.ones((128, BC), NPBF), iden=np.eye(128, dtype=NPBF), dnsc=dnsc)

    percore = []
    for c in range(NCORES):
        xc = x[c * BC:(c + 1) * BC]              # [BC, T, F]
        xpad = np.zeros((BC, T, 512), f32)
        xpad[:, :, :F] = xc
        xpad[:, :, F] = 1.0
        xT = (xpad.reshape(BC, T, 4, 128).transpose(1, 3, 2, 0)
              .reshape(T, 128, 4 * BC))
        percore.append({"xT": np.ascontiguousarray(xT).astype(NPF8)})
    return shared, percore, c_all


_CACHED = {}


def _get_module():
    if "nc" not in _CACHED:
        _CACHED["nc"] = _build_module()
    return _CACHED["nc"]


def kernel(**inputs) -> np.ndarray:
    shared, percore, c_all = host_prep(inputs)
    nc = _get_module()
    in_maps = [{**shared, **percore[c]} for c in range(NCORES)]
    res = run_bass_kernel_spmd(nc, in_maps, core_ids=list(range(NCORES)))
    outs = [res.results[c]["out"].reshape(BC) for c in range(NCORES)]
    out = np.concatenate(outs).astype(np.float32) + np.float32(c_all)
    return out.reshape(B, 1)


# revision 25
# speedup vs baseline: 1.2578x; 1.0431x over previous
"""Trainium2 Bass kernel for the 2-layer GRU discriminator
(B=1024, T=63, F=257, H=512  ->  out [1024, 1]).

Strategy (pure data parallelism over batch, 8 cores x 128 batch each):
  - All weights/activations resident in SBUF; x streamed per timestep.
  - State kept as h [b=128 partitions, H free] in bf16; per-step PE
    transposes produce hT (cast to fp8) used as the matmul stationary
    operand, so gate matmuls run with the (static, SBUF-resident) weight
    matrices as the moving operand at N=512 free-dim.
  - Gate matmuls are fp8e4m3 with perf_mode=DoubleRow: K-chunk pairs are
    packed [128, 2, dim] so each matmul contracts 256 rows (2 fp8
    weights/cell), halving PE streaming time. Weights are pre-scaled by
    WSCALE=16 to stay in fp8's normal range; sigmoids/tanh descale via the
    free `scale=` affine of the ACT instruction.
  - L0's x contraction uses its true K: one DoubleRow matmul covers
    features 0..255; the tail (feature 256 + ones row carrying the fused
    input biases) is a K=2 matmul per gate, issued as row-tiled
    (tile_position) matmuls on distinct 32-row groups so the three gate
    tails plus the K=1 bhh0n bias matmul all stream concurrently.
  - L1's four bias rows are likewise a single concurrent row-tiled quad
    of K=1 matmuls instead of four serial N=512 streams.
  - Gate tail uses h' = z*prev - (z-1)*n: one fused scalar_tensor_tensor
    computes (z-1)*n, eliminating the extra sigmoid(-x) ACT op per layer.
  - The x-part matmuls for step t+1 are issued mid-step t (lookahead) so
    the PE stays busy while the serial sigmoid/tanh gate chain runs -- this
    also keeps the PE HAM clock monitor at full rate.
  - The entire MLP head collapses to out[b] = sum_t v[t]*(c_t . dnn_w) + c0
    (v = w3@w2@w1), accumulated across all 63 steps into one PSUM bank by
    M=1 matmuls against the per-step transposed state.
"""
import numpy as np
import ml_dtypes
from contextlib import ExitStack

import concourse.bass as bass
import concourse.tile as tile
from concourse import bacc, mybir
from concourse.bass_utils import run_bass_kernel_spmd

AF = mybir.ActivationFunctionType
OP = mybir.AluOpType
PM = mybir.MatmulPerfMode
F32 = mybir.dt.float32
BF16 = mybir.dt.bfloat16
FP8 = mybir.dt.float8e4
NPBF = ml_dtypes.bfloat16
NPF8 = ml_dtypes.float8_e4m3

B, T, F, H = 1024, 63, 257, 512
NCORES = 8
BC = B // NCORES          # 128 batch per core
G3 = 3 * H                # 1536
NK = H // 128             # 4 hidden chunks
NKQ = NK // 2             # 2 hidden chunk-pairs (DoubleRow)
WSCALE = 16.0             # fp8 weight pre-scale (descaled in sigmoid/tanh)
DSCALE = 4096.0           # head dnn-weight pre-scale (descaled in out copy)
U_ON_GPSIMD = True        # z*prev on the (otherwise idle) GPSIMD engine


def _dr(ap):
    """[128, 2*X] slice -> [128, 2, X] chunk-pair AP for DoubleRow."""
    return ap.rearrange("p (i b) -> p i b", i=2)


def _build_module():
    nc = bacc.Bacc("TRN2", target_bir_lowering=False, debug=False)

    xT_d = nc.dram_tensor("xT", [T, 128, 4 * BC], FP8, kind="ExternalInput").ap()
    wih0_d = nc.dram_tensor("wih0", [2, 128, 2 * G3], FP8, kind="ExternalInput").ap()
    whh0_d = nc.dram_tensor("whh0", [NKQ, 128, 2 * G3], FP8, kind="ExternalInput").ap()
    wih1_d = nc.dram_tensor("wih1", [NKQ, 128, 2 * G3], FP8, kind="ExternalInput").ap()
    whh1_d = nc.dram_tensor("whh1", [NKQ, 128, 2 * G3], FP8, kind="ExternalInput").ap()
    bw_d = nc.dram_tensor("bw", [128, 3 * H], BF16, kind="ExternalInput").ap()
    onesq_d = nc.dram_tensor("onesq", [128, BC], BF16, kind="ExternalInput").ap()
    iden_d = nc.dram_tensor("iden", [128, 128], BF16, kind="ExternalInput").ap()
    dnsc_d = nc.dram_tensor("dnsc", [NKQ, 128, 2 * 64], FP8, kind="ExternalInput").ap()
    out_d = nc.dram_tensor("out", [1, BC], F32, kind="ExternalOutput").ap()

    with tile.TileContext(nc) as tc, ExitStack() as ctx:
        wp = ctx.enter_context(tc.tile_pool(name="wp", bufs=1, space="SBUF"))
        xp = ctx.enter_context(tc.tile_pool(name="xp", bufs=4, space="SBUF"))
        sp = ctx.enter_context(tc.tile_pool(name="sp", bufs=2, space="SBUF"))
        pg = ctx.enter_context(tc.tile_pool(name="pg", bufs=3, space="PSUM"))
        pt = ctx.enter_context(tc.tile_pool(name="pt", bufs=1, space="PSUM"))

        # --- resident weights (chunk-pair packed for DoubleRow) ---
        wih0 = [wp.tile_from(wih0_d[q], name=f"wih0_{q}") for q in range(2)]
        whh0 = [wp.tile_from(whh0_d[q], name=f"whh0_{q}") for q in range(NKQ)]
        wih1 = [wp.tile_from(wih1_d[q], name=f"wih1_{q}") for q in range(NKQ)]
        whh1 = [wp.tile_from(whh1_d[q], name=f"whh1_{q}") for q in range(NKQ)]
        bw = wp.tile_from(bw_d, name="bw")
        onesq = wp.tile_from(onesq_d, name="onesq")
        iden = wp.tile_from(iden_d, name="iden")
        dnsc = [wp.tile_from(dnsc_d[q], name=f"dnsc_{q}") for q in range(NKQ)]

        def wslice(wtile, g0, g1):
            """[128, 2*G3] pair tile -> [128, 2, g1-g0] moving operand."""
            return wtile.rearrange("p (i g) -> p i g", i=2)[:, :, g0:g1]

        # all 63 transposed c-states parked in SBUF; the tiny head
        # reduction runs as a dense epilogue, freeing a PSUM bank for the
        # L0 gate ring during the loop
        cTall = wp.tile([128, T * H], FP8, name="cTall", tag="cTall")

        aT = None
        cT = None
        a_prev = None
        c_prev = None

        def alloc_g(nm, tag, bufs=None):
            pr = pg.tile([BC, H], F32, name=f"pr{nm}", tag=tag, bufs=bufs)
            pz = pg.tile([BC, H], F32, name=f"pz{nm}", tag=tag, bufs=bufs)
            phn = pg.tile([BC, H], F32, name=f"phn{nm}", tag=tag, bufs=bufs)
            pxn = pg.tile([BC, H], F32, name=f"pxn{nm}", tag=tag, bufs=bufs)
            return pr, pz, phn, pxn

        def dma_x(t):
            xt = xp.tile([128, 4 * BC], FP8, name="xt", tag="xt")
            nc.sync.dma_start(out=xt, in_=xT_d[t])
            return xt

        def l0_xpart(t, g, xt):
            """x-side matmuls for step t into L0 psum gen g=(pr,pz,phn,pxn):
            two DoubleRow pairs per gate (x padded to 512 with a ones row at
            feature 257 carrying the fused input biases), plus a row-tiled
            K=1 matmul for the bhh0n bias."""
            pr, pz, phn, pxn = g
            last = t == 0  # no hidden matmuls at t=0: close groups here
            for q in range(2):
                xq = _dr(xt[:, 2 * q * BC:(2 * q + 2) * BC])
                nc.tensor.matmul(pr, xq, wslice(wih0[q], 0, H),
                                 start=(q == 0), stop=(last and q == 1),
                                 perf_mode=PM.DoubleRow)
                nc.tensor.matmul(pz, xq, wslice(wih0[q], H, 2 * H),
                                 start=(q == 0), stop=(last and q == 1),
                                 perf_mode=PM.DoubleRow)
                nc.tensor.matmul(pxn, xq, wslice(wih0[q], 2 * H, G3),
                                 start=(q == 0), stop=(q == 1),
                                 perf_mode=PM.DoubleRow)
            nc.tensor.matmul(phn, onesq[0:1, :], bw[0:1, 2 * H:3 * H],
                             start=True, stop=last, tile_position=(0, 0))

        # --- gate-chain stages, split for instruction-level interleaving ---
        def gates_sig(g, nm):
            """ACT: r = sig(pr), z = sig(pz)."""
            r = sp.tile([BC, H], BF16, name=f"r_{nm}", tag=f"r_{nm}")
            z = sp.tile([BC, H], BF16, name=f"z_{nm}", tag=f"z_{nm}")
            nc.scalar.activation(out=r, in_=g[0], func=AF.Sigmoid, scale=1.0 / WSCALE)
            nc.scalar.activation(out=z, in_=g[1], func=AF.Sigmoid, scale=1.0 / WSCALE)
            return r, z

        def gates_pre(g, r, z, prev, nm, split=False):
            """DVE: t4 = r*phn + pxn (optionally in H-halves for a shorter
            critical path to the low half);  zc = 1-z;  GPSIMD: u = z*prev."""
            t3 = sp.tile([BC, H], BF16, name=f"t3_{nm}", tag=f"t3_{nm}")
            t4 = sp.tile([BC, H], BF16, name=f"t4_{nm}", tag=f"t4_{nm}")
            zc = sp.tile([BC, H], BF16, name=f"zc_{nm}", tag=f"zc_{nm}")
            halves = (slice(0, H // 2), slice(H // 2, H)) if split else (slice(0, H),)
            u = None
            if prev is not None:
                u = sp.tile([BC, H], BF16, name=f"u_{nm}", tag=f"u_{nm}")
            for i, hs in enumerate(halves):
                nc.vector.tensor_tensor(out=t3[:, hs], in0=r[:, hs],
                                        in1=g[2][:, hs], op=OP.mult)
                nc.vector.tensor_tensor(out=t4[:, hs], in0=t3[:, hs],
                                        in1=g[3][:, hs], op=OP.add)
                if i == 0:
                    nc.vector.tensor_scalar(out=zc, in0=z, scalar1=-1.0,
                                            scalar2=1.0, op0=OP.mult, op1=OP.add)
                    if prev is not None:
                        ueng = nc.gpsimd if U_ON_GPSIMD else nc.vector
                        ueng.tensor_tensor(out=u, in0=z, in1=prev, op=OP.mult)
            return t4, zc, u

        def gates_tanh(t4, nm, split=False, half=None):
            if half is None:
                nn_t = sp.tile([BC, H], BF16, name=f"n_{nm}", tag=f"n_{nm}")
                hss = (slice(0, H // 2), slice(H // 2, H)) if split else (slice(0, H),)
                for hs in hss:
                    nc.scalar.activation(out=nn_t[:, hs], in_=t4[:, hs],
                                         func=AF.Tanh, scale=1.0 / WSCALE)
                return nn_t
            return None

        def gates_tail(zc, nn_t, u, nm, split=False):
            """DVE: h' = (1-z)*n + u, optionally per H-half so the low half
            of the state (and its transpose+copy) lands early."""
            hnew = sp.tile([BC, H], BF16, name=f"h_{nm}", tag=f"h_{nm}")
            hss = (slice(0, H // 2), slice(H // 2, H)) if split else (slice(0, H),)
            for hs in hss:
                if u is None:
                    nc.vector.tensor_tensor(out=hnew[:, hs], in0=zc[:, hs],
                                            in1=nn_t[:, hs], op=OP.mult)
                else:
                    t6 = sp.tile([BC, H // 2 if split else H], BF16,
                                 name=f"t6_{nm}", tag=f"t6_{nm}_{hs.start}")
                    nc.vector.tensor_tensor(out=t6, in0=zc[:, hs],
                                            in1=nn_t[:, hs], op=OP.mult)
                    nc.vector.tensor_tensor(out=hnew[:, hs], in0=t6,
                                            in1=u[:, hs], op=OP.add)
            return hnew

        def tr_mms(h, nm):
            """[BC, H] SBUF bf16 -> [128, H] PSUM f32 transposed chunks.
            Regular matmul h_chunk^T @ I == transpose; unlike transpose-mode
            it counts as PE-busy for the HAM clock monitor."""
            ptr = pt.tile([128, H], F32, name=f"ptr_{nm}", tag="tr")
            for k in range(NK):
                nc.tensor.matmul(
                    ptr[:, k * 128:(k + 1) * 128],
                    h[:, k * 128:(k + 1) * 128],
                    iden,
                    start=True, stop=True,
                )
            return ptr

        def tr_copy(ptr, nm, eng, dst=None):
            """PSUM f32 -> SBUF fp8, per chunk-pair (first pair ready early).
            dst overrides the output tile (used to park cT in cTall)."""
            hT = dst if dst is not None else sp.tile(
                [128, H], FP8, name=f"hT_{nm}", tag=f"hT_{nm}")
            for q in range(2):
                sl = slice(q * 256, (q + 1) * 256)
                if eng == "act":
                    nc.scalar.activation(out=hT[:, sl], in_=ptr[:, sl], func=AF.Copy)
                else:
                    nc.vector.tensor_copy(out=hT[:, sl], in_=ptr[:, sl])
            return hT

        def l0_hidden(g, aT):
            """L0 hidden-side DR matmuls: r, n, z order (t3 needs phn early)."""
            for gs, pdst in ((0, g[0]), (2 * H, g[2]), (H, g[1])):
                for q in range(NKQ):
                    aq = _dr(aT[:, 2 * q * 128:(2 * q + 2) * 128])
                    nc.tensor.matmul(pdst, aq, wslice(whh0[q], gs, gs + H),
                                     start=False, stop=(q == NKQ - 1),
                                     perf_mode=PM.DoubleRow)

        def l1_aside(g1, aT, t):
            """L1 x-side (a-state) matmuls; opens pr1/pz1/pxn1 and appends
            their K=1 bias rows (row-tiled) — the bias quad is folded into
            these groups so no early PSUM-slot wait stalls the PE queue."""
            for gs, pdst, bsl, tp in ((0, g1[0], bw[0:1, 0:H], (0, 0)),
                                      (H, g1[1], bw[32:33, 0:H], (32, 0)),
                                      (2 * H, g1[3], bw[0:1, H:2 * H], (0, 0))):
                for q in range(NKQ):
                    aq = _dr(aT[:, 2 * q * 128:(2 * q + 2) * 128])
                    nc.tensor.matmul(pdst, aq, wslice(wih1[q], gs, gs + H),
                                     start=(q == 0), stop=False,
                                     perf_mode=PM.DoubleRow)
                row = onesq[tp[0]:tp[0] + 1, :] if tp[0] else onesq[0:1, :]
                bias_stop = (t == 0 and gs != 2 * H) or gs == 2 * H
                nc.tensor.matmul(pdst, row, bsl, start=False, stop=bias_stop,
                                 tile_position=tp)

        def l1_hidden(g1, cT):
            """L1 hidden-side DR matmuls: r, n, z; closes pr1/pz1/phn1.
            phn1's group opens with its K=1 bhh1n bias matmul."""
            nc.tensor.matmul(g1[2], onesq[64:65, :], bw[64:65, 0:H],
                             start=True, stop=False, tile_position=(64, 0))
            for gs, pdst in ((0, g1[0]), (2 * H, g1[2]), (H, g1[1])):
                for q in range(NKQ):
                    cq = _dr(cT[:, 2 * q * 128:(2 * q + 2) * 128])
                    nc.tensor.matmul(pdst, cq, wslice(whh1[q], gs, gs + H),
                                     start=False, stop=(q == NKQ - 1),
                                     perf_mode=PM.DoubleRow)

        def head_mms(t, head, stop=False):
            ct = cTall[:, t * H:(t + 1) * H]
            for q in range(NKQ):
                hl = dnsc[q].rearrange("p (i t) -> p i t", i=2)[:, :, t:t + 1]
                nc.tensor.matmul(head, hl,
                                 _dr(ct[:, 2 * q * 128:(2 * q + 2) * 128]),
                                 start=(t == 0 and q == 0),
                                 stop=(stop and q == NKQ - 1),
                                 perf_mode=PM.DoubleRow)

        # ---- prologue: step 0 x-side + a-gates(0), lookahead x for 1,2 ----
        xts = {t: dma_x(t) for t in range(min(3, T))}
        g0_cur = alloc_g("0", "g0")          # gen 0
        l0_xpart(0, g0_cur, xts.pop(0))
        g1_cur = alloc_g("1", "g1")          # gen 0 (filled at F in iter 0)
        r0, z0 = gates_sig(g0_cur, "a")
        t40, zc0, u0 = gates_pre(g0_cur, r0, z0, None, "a")
        n0 = gates_tanh(t40, "a")
        a_state = gates_tail(zc0, n0, u0, "a")
        ptr_a = tr_mms(a_state, "a")
        aT = tr_copy(ptr_a, "a", "act")
        g0_cur = alloc_g("0", "g0")          # gen 1
        if T > 1:
            l0_xpart(1, g0_cur, xts.pop(1))

        c_state = None      # c(t-1) state: prev for c-gates AND pending tr-c
        c_new = None
        cT = None
        # Deep software pipeline, anchored on the layer-0 chain. The
        # transpose+copy of a(t+1) happens at the END of iteration t (right
        # after h'a(t+1) is produced), so iteration t+1's L0h matmuls start
        # with zero wait -- the PE never idles at the iteration boundary and
        # the HAM clock monitor stays at full rate.
        for t in range(T):
            # A: L0 hidden for t+1 (aT(t) transposed+copied last iter)
            if t + 1 < T:
                l0_hidden(g0_cur, aT)
                r0, z0 = gates_sig(g0_cur, "a")
                t40, zc0, u0 = gates_pre(g0_cur, r0, z0, a_state, "a",
                                         split=True)
            # F: L1 a-side for t (alloc for t happened last iter / prologue)
            l1_aside(g1_cur, aT, t)
            # H: transpose c(t-1) (from the end of last iter); casts on DVE
            # park the result in cTall for the head epilogue
            if t > 0:
                ptr_c = tr_mms(c_new, "c")
                cT = tr_copy(ptr_c, "c", "dve",
                             dst=cTall[:, (t - 1) * H:t * H])
            # tanh0(t+1) in halves
            if t + 1 < T:
                n0 = gates_tanh(t40, "a", split=True)
            # D: x-side lookahead for t+2
            if t + 2 < T:
                if t + 3 < T:
                    xts[t + 3] = dma_x(t + 3)
                g0_cur = alloc_g("0", "g0")
                l0_xpart(t + 2, g0_cur, xts.pop(t + 2))
            # a-tail: h'a(t+1), low half first
            if t + 1 < T:
                a_next = gates_tail(zc0, n0, u0, "a", split=True)
                # E: transpose + copy a(t+1) now — before L1h in the PE
                # queue and before the c-sigmoids in the ACT queue
                ptr_a = tr_mms(a_next, "a")
                aT_next = tr_copy(ptr_a, "a", "act")
            # L1 hidden for t (needs cT(t-1))
            if t > 0:
                l1_hidden(g1_cur, cT)
            else:
                # t=0: no c-state; phn1(0) is its bias alone
                nc.tensor.matmul(g1_cur[2], onesq[64:65, :], bw[64:65, 0:H],
                                 start=True, stop=True, tile_position=(64, 0))
            # c-gates(t)
            r1, z1 = gates_sig(g1_cur, "c")
            t41, zc1, u1 = gates_pre(g1_cur, r1, z1, c_state, "c")
            n1 = gates_tanh(t41, "c")
            c_next = gates_tail(zc1, n1, u1, "c")
            # g1 banks for t+1: slots just freed by the c-gates above
            if t + 1 < T:
                g1_next = alloc_g("1", "g1")
            # rotate
            c_state = c_new = c_next
            if t + 1 < T:
                a_state = a_next
                aT = aT_next
                g1_cur = g1_next

        # ---- epilogue: transpose c(T-1), then the dense head reduction
        # over all parked cT states (PSUM banks are free now) ----
        ptr_c = tr_mms(c_new, "c")
        tr_copy(ptr_c, "c", "dve", dst=cTall[:, (T - 1) * H:T * H])
        head = pg.tile([1, BC], F32, name="head", tag="g1")
        for t in range(T):
            head_mms(t, head, stop=(t == T - 1))
        out_sb = sp.tile([1, BC], F32, name="out_sb", tag="out_sb")
        nc.scalar.activation(out=out_sb, in_=head, func=AF.Copy, scale=1.0 / DSCALE)
        nc.sync.dma_start(out=out_d, in_=out_sb)

    # legalize sem waits (>=2 waits per matmul is a codegen error) etc.
    nc.compile()
    return nc


def _pack_pairs(wt):
    """[512, G3] (contraction-major) -> [NQ, 128, 2*G3] chunk-pair tiles:
    out[q][p, i*G3+g] = wt[(2q+i)*128 + p, g]"""
    nq = wt.shape[0] // 256
    return np.ascontiguousarray(
        wt.reshape(nq, 2, 128, -1).transpose(0, 2, 1, 3).reshape(nq, 128, -1))


def host_prep(inputs):
    f32 = np.float32
    x = np.asarray(inputs["x"], f32)
    w_ih0, w_hh0 = np.asarray(inputs["w_ih0"], f32), np.asarray(inputs["w_hh0"], f32)
    b_ih0, b_hh0 = np.asarray(inputs["b_ih0"], f32), np.asarray(inputs["b_hh0"], f32)
    w_ih1, w_hh1 = np.asarray(inputs["w_ih1"], f32), np.asarray(inputs["w_hh1"], f32)
    b_ih1, b_hh1 = np.asarray(inputs["b_ih1"], f32), np.asarray(inputs["b_hh1"], f32)
    dnn_w, dnn_b = np.asarray(inputs["dnn_w"], f32), np.asarray(inputs["dnn_b"], f32)
    w1, b1 = np.asarray(inputs["w1"], f32), np.asarray(inputs["b1"], f32)
    w2, b2 = np.asarray(inputs["w2"], f32), np.asarray(inputs["b2"], f32)
    w3, b3 = np.asarray(inputs["w3"], f32), np.asarray(inputs["b3"], f32)

    # L0 input weights: features 0..255 as one DoubleRow pair chunk; the
    # tail tile wtl carries feature 256 (row 0 of each pair) and the fused
    # biases (row 1): b_ih0+b_hh0 for r/z, b_ih0 for n; plus bhh0n at row 96.
    wih0f = np.zeros((512, G3), f32)
    wih0f[:F] = w_ih0.T
    wih0f[F] = np.concatenate([(b_ih0 + b_hh0)[:2 * H], b_ih0[2 * H:]])
    wih0 = _pack_pairs(wih0f * WSCALE).astype(NPF8)   # [2, 128, 2*G3]
    whh0 = _pack_pairs(w_hh0.T * WSCALE).astype(NPF8)
    wih1 = _pack_pairs(w_ih1.T * WSCALE).astype(NPF8)
    whh1 = _pack_pairs(w_hh1.T * WSCALE).astype(NPF8)

    # L1 bias rows for the row-tiled quad: b1r/b1z (=b_ih1+b_hh1), bhh1n, bih1n
    b1g = b_ih1 + b_hh1
    bw = np.zeros((128, 3 * H), f32)
    bw[0, :H] = b1g[:H]
    bw[32, :H] = b1g[H:2 * H]
    bw[64, :H] = b_hh1[2 * H:]
    bw[0, H:2 * H] = b_ih1[2 * H:]
    bw[0, 2 * H:] = b_hh0[2 * H:]
    bw = (bw * WSCALE).astype(NPBF)

    v = (w3 @ w2 @ w1)[0]
    # chunk-pair packed for DoubleRow, inner dim padded 63->64 so the
    # pair-dim byte step (64) satisfies the fp8-DR step%16==0 ISA rule
    dfull = np.zeros((H, 64), f32)
    dfull[:, :T] = dnn_w[0][:, None] * v[None, :] * DSCALE
    dnsc = np.ascontiguousarray(
        dfull.reshape(NKQ, 2, 128, 64).transpose(0, 2, 1, 3)
        .reshape(NKQ, 128, 2 * 64)).astype(NPF8)
    c_all = float(v.sum() * dnn_b[0] + (w3 @ w2 @ b1)[0] + (w3 @ b2)[0] + b3[0])

    shared = dict(
        wih0=wih0, whh0=whh0, wih1=wih1, whh1=whh1, bw=bw,
        onesq=np.ones((128, BC), NPBF), iden=np.eye(128, dtype=NPBF), dnsc=dnsc)

    percore = []
    for c in range(NCORES):
        xc = x[c * BC:(c + 1) * BC]              # [BC, T, F]
        xpad = np.zeros((BC, T, 512), f32)
        xpad[:, :, :F] = xc
        xpad[:, :, F] = 1.0
        xT = (xpad.reshape(BC, T, 4, 128).transpose(1, 3, 2, 0)
              .reshape(T, 128, 4 * BC))
        percore.append({"xT": np.ascontiguousarray(xT).astype(NPF8)})
    return shared, percore, c_all


_CACHED = {}


def _get_module():
    if "nc" not in _CACHED:
        _CACHED["nc"] = _build_module()
    return _CACHED["nc"]


def kernel(**inputs) -> np.ndarray:
    shared, percore, c_all = host_prep(inputs)
    nc = _get_module()
    in_maps = [{**shared, **percore[c]} for c in range(NCORES)]
    res = run_bass_kernel_spmd(nc, in_maps, core_ids=list(range(NCORES)))
    outs = [res.results[c]["out"].reshape(BC) for c in range(NCORES)]
    out = np.concatenate(outs).astype(np.float32) + np.float32(c_all)
    return out.reshape(B, 1)
